# revision 1
# baseline (speedup 1.0000x reference)
"""Trainium2 Bass kernel for nn_AttentionManifold (SPD manifold attention).

For each of bs*m=2048 SPD matrices X (100x100): Q/K/V = W^T X W (64x64),
logQ/K/V = matrix log, log-Euclidean attention (Frobenius distances ->
scores -> softmax over K index), mixed = prob-weighted sum of logV,
out = matrix exp(mixed).

Matrix log via tuned Newton-Schulz sqrt chain (4 levels, R = (A/16)^(1/16),
log A = 16 log R + log16*I; the global log16*I terms cancel in the
attention distances and fold into a final *16 output scale), log R via a
degree-14 series (Paterson-Stockmeyer), exp via scaling-squaring (k=5,
degree-7 Taylor).  Q/K paths use fp16 matmuls (scores are insensitive);
V path, congruence mm1 and exp use fp32 matmuls.

Sharding: pure data parallelism, bs=32 -> 4 samples per NeuronCore.
"""
import numpy as np
from contextlib import ExitStack

C_NORM = 16.0
SCHED = [
    [(24.871321977, -35.245186442),
     (1.605560380, -0.024430481),
     (1.595838197, -0.060908024),
     (1.576384611, -0.143218467),
     (1.543497701, -0.291162661),
     (1.511244305, -0.443655343),
     (1.5, -0.5), (1.5, -0.5)],
    [(6.228647233, -6.864010667),
     (1.554009519, -0.242273245),
     (1.518749014, -0.406941447),
     (1.5, -0.5), (1.5, -0.5), (1.5, -0.5)],
    [(3.051424190, -2.460263319),
     (1.508484255, -0.457724181),
     (1.5, -0.5), (1.5, -0.5)],
    [(2.128257338, -1.230895381),
     (1.5, -0.5), (1.5, -0.5)],
]
EXP_DEG = 4
EXP_SQ = 4
DEBUG = False

BS, M, DIN, DOUT = 32, 64, 100, 64
NCORES = 8
NSAMP = BS // NCORES
NP_PAIR = M // 2
PAIR_BATCH = 4
NBATCH = NP_PAIR // PAIR_BATCH


def _flat_sched(nlevels=4):
    out = []
    for steps in SCHED[:nlevels]:
        for j, (a, b) in enumerate(steps):
            out.append((j == 0, a, b))
    return out


def emit_kernel(nc, tc, ctx, x_ap, wq_ap, wk_ap, wv_ap, out_ap, nsamp=NSAMP, taps=None):
    def tap(name, t):
        if taps is not None and name in taps:
            nc.sync.dma_start(out=taps[name], in_=t)
    import concourse.mybir as mybir
    from concourse.bass import ds, ts
    from concourse.masks import make_identity

    f32 = mybir.dt.float32
    f16 = mybir.dt.float16
    AX = mybir.AxisListType
    OP = mybir.AluOpType
    ACT = mybir.ActivationFunctionType
    WB = PAIR_BATCH * 64

    const = ctx.enter_context(tc.tile_pool(name="const", bufs=1))
    work = ctx.enter_context(tc.tile_pool(name="work", bufs=2))
    big = ctx.enter_context(tc.tile_pool(name="big", bufs=1))
    init2 = ctx.enter_context(tc.tile_pool(name="init2", bufs=2))
    logs = ctx.enter_context(tc.tile_pool(name="logs", bufs=2))
    chain = ctx.enter_context(tc.tile_pool(name="chain", bufs=3))
    ser = ctx.enter_context(tc.tile_pool(name="ser", bufs=1))
    chainP = ctx.enter_context(tc.tile_pool(name="chainP", bufs=2))
    ps_small = ctx.enter_context(tc.tile_pool(name="ps_s", bufs=1, space="PSUM"))
    ps_big = ctx.enter_context(tc.tile_pool(name="ps_b", bufs=2, space="PSUM"))
    ps_mid = ctx.enter_context(tc.tile_pool(name="ps_m", bufs=1, space="PSUM"))

    # ---------------- constants ----------------
    W3 = const.tile([DIN, 3 * DOUT], f32)
    nc.sync.dma_start(out=W3[:, 0:DOUT], in_=wq_ap)
    nc.sync.dma_start(out=W3[:, DOUT:2 * DOUT], in_=wk_ap)
    nc.sync.dma_start(out=W3[:, 2 * DOUT:3 * DOUT], in_=wv_ap)
    WQh = const.tile([DIN, DOUT], f16)
    WKh = const.tile([DIN, DOUT], f16)
    nc.vector.tensor_copy(out=WQh, in_=W3[:, 0:DOUT])
    nc.vector.tensor_copy(out=WKh, in_=W3[:, DOUT:2 * DOUT])

    IREP16 = const.tile([128, 64], f16)
    IREP32 = const.tile([128, 64], f32)
    for t in (IREP16, IREP32):
        make_identity(nc, t[0:64, :])
        make_identity(nc, t[64:128, :])
    # widened identity / block-coefficient tiles [128, WB]
    IW = {}
    for dt_, rep, tag in ((f16, IREP16, "16"), (f32, IREP32, "32")):
        w = const.tile([128, WB], dt_, tag=f"IW{tag}")
        for p in range(PAIR_BATCH):
            nc.vector.tensor_copy(out=w[:, ts(p, 64)], in_=rep)
        IW[tag] = w
    IWD = {}
    for dt_, rep, tag in ((f16, IREP16, "16"), (f32, IREP32, "32")):
        w = const.tile([128, PAIR_BATCH * 128], dt_, tag=f"IWD{tag}")
        for p in range(2 * PAIR_BATCH):
            nc.vector.tensor_copy(out=w[:, ts(p, 64)], in_=rep)
        IWD[tag] = w
    cI = {}
    for tag in ("16", "32"):
        for k in (4, 8, 12):
            dt_ = f16 if tag == "16" else f32
            t = const.tile([128, WB], dt_, tag=f"cI{tag}_{k}")
            nc.vector.tensor_scalar_mul(t, IW[tag], 1.0 / k)
            cI[(tag, k)] = t

    ones_col = const.tile([64, 1], f32)
    nc.vector.memset(ones_col, 1.0)
    ones_col_h = const.tile([64, 1], f16)
    nc.vector.memset(ones_col_h, 16.0)      # folds the /16 exp prescale
    ones_row = const.tile([1, 64], f32)
    nc.vector.memset(ones_row, 1.0)
    bias_ln = const.tile([64, 1], f32)
    nc.vector.memset(bias_ln, 1.0 + 64e-6)
    bias_one = const.tile([64, 1], f32)
    nc.vector.memset(bias_one, 1.0)

    FS4 = _flat_sched(4)
    FS3 = _flat_sched(3)

    def mm_pairs(out_ps, lhs_t, rhs_t, ncols=64):
        for p in range(PAIR_BATCH):
            for h in (0, 1):
                nc.tensor.matmul(
                    out_ps[h * 64:h * 64 + 64, ts(p, ncols)],
                    lhs_t[h * 64:h * 64 + 64, ts(p, 64)],
                    rhs_t[h * 64:h * 64 + 64, ts(p, ncols)],
                    start=True, stop=True)

    def chain_and_series(init_t, dt_, tag, flat_t, b):
        # generator: yields after each NS step so Q/K/V emission interleaves
        # V runs level 1 in fp32 (ill-conditioned state), then fp16.
        # Q/K use 3 sqrt levels (log scale 8), V uses 4 (scale 16).
        # V also uses 3 sqrt levels: emulator-validated better than 4
        FS = FS3
        lscale = -8.0
        irep = IW["16" if dt_ == f16 else "32"]
        ctag = "16" if dt_ == f16 else "32"
        if True:
            cs = ds(b * WB, WB)
            # state quad [Y | Yt | Z | Zt] per pair, 256 cols each
            SQ = chain.tile([128, PAIR_BATCH * 256], dt_, tag=f"SQ{tag}")
            sq4 = SQ.rearrange("p (n f c) -> p n f c", f=4, c=64)
            iv = init_t[:, cs].rearrange("p (n c) -> p n c", c=64)
            nc.vector.tensor_copy(out=sq4[:, :, 0, :], in_=iv)
            nc.vector.tensor_copy(out=sq4[:, :, 1, :], in_=iv)
            ir3 = irep.rearrange("p (n c) -> p n c", c=64)

            def qmm(out_ps, oslice, lhs4, li, rhs4, ri):
                for p in range(PAIR_BATCH):
                    for h in (0, 1):
                        nc.tensor.matmul(
                            out_ps[h * 64:h * 64 + 64, p * oslice[1] + oslice[0] * 64:
                                   p * oslice[1] + oslice[0] * 64 + 64],
                            lhs4[h * 64:h * 64 + 64, p * 256 + li * 64:p * 256 + li * 64 + 64],
                            rhs4[h * 64:h * 64 + 64, p * 256 + ri * 64:p * 256 + ri * 64 + 64] if ri is not None
                            else rhs4[h * 64:h * 64 + 64, ts(p, 64)],
                            start=True, stop=True)

            for k_idx, (lvl_start, al, be) in enumerate(FS):
                if lvl_start and k_idx == 8 and dt_ == f32:
                    # V-path precision drop: fp32 -> fp16 from level 2 on
                    dt_ = f16
                    irep = IW["16"]
                    ctag = "16"
                    ir3 = irep.rearrange("p (n c) -> p n c", c=64)
                    SQn = chain.tile([128, PAIR_BATCH * 256], dt_, tag=f"SQ{tag}")
                    sqn4 = SQn.rearrange("p (n f c) -> p n f c", f=4, c=64)
                    nc.vector.tensor_copy(out=sqn4[:, :, 0, :], in_=sq4[:, :, 0, :])
                    nc.vector.tensor_copy(out=sqn4[:, :, 1, :], in_=sq4[:, :, 1, :])
                    SQ, sq4 = SQn, sqn4
                if lvl_start:
                    # Z = I here, so W = Y and the level-start step needs no
                    # W-matmuls: P = aI + b*Y (from SBUF), and Z' = P Z = P.
                    Pb = chainP.tile([128, 2 * WB], dt_, tag=f"Pb{tag}")
                    pb3 = Pb.rearrange("p (n f c) -> p n f c", f=2, c=64)
                    nc.scalar.activation(out=pb3, in_=sq4[:, :, 0:2, :],
                                         func=ACT.Copy, bias=0.0, scale=be)
                    nc.vector.scalar_tensor_tensor(
                        out=Pb, in0=IWD[ctag], scalar=al,
                        in1=Pb, op0=OP.mult, op1=OP.add)
                else:
                    # W = Zt^T Y ; Wt = Y^T Zt
                    psA = ps_big.tile([128, PAIR_BATCH * 128], mybir.dt.float32, tag="psA")
                    qmm(psA, (0, 128), SQ, 3, SQ, 0)
                    qmm(psA, (1, 128), SQ, 0, SQ, 3)
                    Pb = chainP.tile([128, 2 * WB], dt_, tag=f"Pb{tag}")
                    nc.scalar.activation(out=Pb, in_=psA, func=ACT.Copy,
                                         bias=0.0, scale=be)
                    nc.vector.scalar_tensor_tensor(
                        out=Pb, in0=IWD[ctag], scalar=al,
                        in1=Pb, op0=OP.mult, op1=OP.add)
                # P = Pb[...,0], Pt = Pb[...,1]
                # yield here so the psB/psC matmuls are emitted a round later:
                # the other two chains' psA matmuls fill the PE queue while
                # this chain's Pb stt completes (avoids in-order head-block)
                yield
                psB = ps_big.tile([128, PAIR_BATCH * 128], mybir.dt.float32, tag="psB")
                for p in range(PAIR_BATCH):
                    for h in (0, 1):
                        hs = slice(h * 64, h * 64 + 64)
                        yt = SQ[hs, p * 256 + 64:p * 256 + 128]
                        pp = Pb[hs, p * 128:p * 128 + 64]
                        nc.tensor.matmul(psB[hs, p * 128:p * 128 + 64], yt, pp,
                                         start=True, stop=True)
                        nc.tensor.matmul(psB[hs, p * 128 + 64:p * 128 + 128], pp, yt,
                                         start=True, stop=True)
                SQ2 = chain.tile([128, PAIR_BATCH * 256], dt_, tag=f"SQ{tag}")
                sq24 = SQ2.rearrange("p (n f c) -> p n f c", f=4, c=64)
                psBr = psB.rearrange("p (n f c) -> p n f c", f=2, c=64)
                nc.scalar.activation(out=sq24[:, :, 0:2, :], in_=psBr,
                                     func=ACT.Copy, bias=0.0, scale=1.0)
                if lvl_start:
                    nc.vector.tensor_copy(
                        out=sq24[:, :, 2:4, :],
                        in_=Pb.rearrange("p (n f c) -> p n f c", f=2, c=64))
                else:
                    psC = ps_big.tile([128, PAIR_BATCH * 128], mybir.dt.float32, tag="psC")
                    for p in range(PAIR_BATCH):
                        for h in (0, 1):
                            hs = slice(h * 64, h * 64 + 64)
                            z = SQ[hs, p * 256 + 128:p * 256 + 192]
                            zt = SQ[hs, p * 256 + 192:p * 256 + 256]
                            pt = Pb[hs, p * 128 + 64:p * 128 + 128]
                            nc.tensor.matmul(psC[hs, p * 128:p * 128 + 64], pt, z,
                                             start=True, stop=True)
                            nc.tensor.matmul(psC[hs, p * 128 + 64:p * 128 + 128], z, pt,
                                             start=True, stop=True)
                    psCr = psC.rearrange("p (n f c) -> p n f c", f=2, c=64)
                    nc.vector.tensor_copy(out=sq24[:, :, 2:4, :], in_=psCr)
                SQ, sq4 = SQ2, sq24
                yield
            # R = (Y + Yt)/2 ; E = I - R
            E = ser.tile([128, WB], dt_, tag=f"E{tag}")
            e3 = E.rearrange("p (n c) -> p n c", c=64)
            nc.vector.tensor_add(e3, sq4[:, :, 0, :], sq4[:, :, 1, :])
            nc.vector.scalar_tensor_tensor(
                out=e3, in0=e3, scalar=-0.5,
                in1=ir3, op0=OP.mult, op1=OP.add)
            psE = ps_mid.tile([128, WB], mybir.dt.float32, tag="ps2")
            mm_pairs(psE, E, E)
            E2 = ser.tile([128, WB], dt_, tag=f"E2{tag}")
            nc.vector.tensor_copy(out=E2, in_=psE)
            psE3 = ps_mid.tile([128, WB], mybir.dt.float32, tag="ps2")
            mm_pairs(psE3, E2, E)
            E3 = ser.tile([128, WB], dt_, tag=f"E3{tag}")
            nc.vector.tensor_copy(out=E3, in_=psE3)
            yield
            psE4 = ps_mid.tile([128, WB], mybir.dt.float32, tag="ps2")
            mm_pairs(psE4, E2, E2)
            E4 = ser.tile([128, WB], dt_, tag=f"E4{tag}")
            nc.vector.tensor_copy(out=E4, in_=psE4)
            B = ser.tile([128, WB], dt_, tag=f"B{tag}")
            nc.vector.scalar_tensor_tensor(out=B, in0=E, scalar=1.0 / 13, in1=cI[(ctag, 12)], op0=OP.mult, op1=OP.add)
            nc.vector.scalar_tensor_tensor(out=B, in0=E2, scalar=1.0 / 14, in1=B, op0=OP.mult, op1=OP.add)
            psH = ps_mid.tile([128, WB], mybir.dt.float32, tag="ps2")
            mm_pairs(psH, E4, B)
            H = ser.tile([128, WB], dt_, tag=f"B{tag}")
            nc.vector.scalar_tensor_tensor(out=H, in0=E, scalar=1.0 / 9, in1=cI[(ctag, 8)], op0=OP.mult, op1=OP.add)
            nc.vector.scalar_tensor_tensor(out=H, in0=E2, scalar=1.0 / 10, in1=H, op0=OP.mult, op1=OP.add)
            nc.vector.scalar_tensor_tensor(out=H, in0=E3, scalar=1.0 / 11, in1=H, op0=OP.mult, op1=OP.add)
            yield
            Hs = ser.tile([128, WB], dt_, tag=f"Hs{tag}")
            nc.vector.tensor_copy(out=Hs, in_=psH)
            nc.vector.tensor_add(H, H, Hs)
            psH2 = ps_mid.tile([128, WB], mybir.dt.float32, tag="ps2")
            mm_pairs(psH2, E4, H)
            H2 = ser.tile([128, WB], dt_, tag=f"B{tag}")
            nc.vector.scalar_tensor_tensor(out=H2, in0=E, scalar=1.0 / 5, in1=cI[(ctag, 4)], op0=OP.mult, op1=OP.add)
            nc.vector.scalar_tensor_tensor(out=H2, in0=E2, scalar=1.0 / 6, in1=H2, op0=OP.mult, op1=OP.add)
            nc.vector.scalar_tensor_tensor(out=H2, in0=E3, scalar=1.0 / 7, in1=H2, op0=OP.mult, op1=OP.add)
            Hs2 = ser.tile([128, WB], dt_, tag=f"Hs{tag}")
            nc.vector.tensor_copy(out=Hs2, in_=psH2)
            nc.vector.tensor_add(H2, H2, Hs2)
            psH3 = ps_mid.tile([128, WB], mybir.dt.float32, tag="ps2")
            mm_pairs(psH3, E4, H2)
            B0 = ser.tile([128, WB], dt_, tag=f"B{tag}")
            nc.vector.tensor_scalar_mul(B0, E2, 0.5)
            nc.vector.scalar_tensor_tensor(out=B0, in0=E3, scalar=1.0 / 3, in1=B0, op0=OP.mult, op1=OP.add)
            nc.vector.tensor_add(B0, B0, E)
            Hs3 = ser.tile([128, WB], dt_, tag=f"Hs{tag}")
            nc.vector.tensor_copy(out=Hs3, in_=psH3)
            nc.vector.tensor_add(B0, B0, Hs3)
            LS = logs.tile([128, WB], flat_t.dtype, tag=f"LS{tag}")
            nc.scalar.activation(out=LS, in_=B0, func=ACT.Copy,
                                 bias=0.0, scale=lscale)
            flat3 = flat_t.rearrange("p (n two c) -> p n two c", two=2, c=64)
            nc.vector.tensor_copy(
                out=flat3[:, ds(b * PAIR_BATCH, PAIR_BATCH), 0, :],
                in_=LS[0:64, :].rearrange("p (n c) -> p n c", c=64))
            nc.gpsimd.dma_start(
                out=flat3[:, ds(b * PAIR_BATCH, PAIR_BATCH), 1, :],
                in_=LS[64:128, :].rearrange("p (n c) -> p n c", c=64))

    # ======================== per-sample pipeline ========================
    for s in range(nsamp):
        initQ = init2.tile([128, NP_PAIR * 64], f16, tag="initQ")
        initK = init2.tile([128, NP_PAIR * 64], f16, tag="initK")
        initV = init2.tile([128, NP_PAIR * 64], f32, tag="initV")
        oddQ = init2.tile([64, NP_PAIR * 64], f16, tag="oddQ")
        oddK = init2.tile([64, NP_PAIR * 64], f16, tag="oddK")
        oddV = init2.tile([64, NP_PAIR * 64], f32, tag="oddV")

        for it in range(M):
            if it % 16 == 0:
                xbuf = work.tile([DIN, 16 * DIN], f32, tag="xbuf")
                nc.sync.dma_start(
                    out=xbuf.rearrange("p (i c) -> p i c", c=DIN),
                    in_=x_ap[s, ds(it, 16)].rearrange("i p c -> p i c"))
            p1 = ps_mid.tile([DIN, 3 * DOUT], mybir.dt.float32, tag="ps2")
            nc.tensor.matmul(p1, xbuf[:, ts(it % 16, DIN)], W3, start=True, stop=True)
            P1qk = work.tile([DIN, 2 * DOUT], f16, tag="p1qk")
            nc.vector.tensor_copy(out=P1qk, in_=p1[:, 0:2 * DOUT])
            P1v = work.tile([DIN, DOUT], f32, tag="p1v")
            nc.vector.tensor_copy(out=P1v, in_=p1[:, 2 * DOUT:3 * DOUT])
            pqkv = ps_small.tile([64, 192], mybir.dt.float32, tag="small")
            nc.tensor.matmul(pqkv[:, 0:64], WQh, P1qk[:, 0:DOUT], start=True, stop=True)
            nc.tensor.matmul(pqkv[:, 64:128], WKh, P1qk[:, DOUT:2 * DOUT], start=True, stop=True)
            nc.tensor.matmul(pqkv[:, 128:192], W3[:, 2 * DOUT:3 * DOUT], P1v, start=True, stop=True)
            pr, h = it // 2, it % 2
            for ci, (init_t, odd_t) in enumerate(((initQ, oddQ), (initK, oddK), (initV, oddV))):
                src = pqkv[:, ci * 64:(ci + 1) * 64]
                if h == 0:
                    nc.scalar.activation(out=init_t[0:64, ts(pr, 64)], in_=src,
                                         func=ACT.Copy, bias=0.0, scale=1.0 / C_NORM)
                else:
                    nc.scalar.activation(out=odd_t[:, ts(pr, 64)], in_=src,
                                         func=ACT.Copy, bias=0.0, scale=1.0 / C_NORM)
        for init_t, odd_t in ((initQ, oddQ), (initK, oddK), (initV, oddV)):
            nc.gpsimd.dma_start(out=init_t[64:128, :], in_=odd_t)

        flatQ = big.tile([64, M * 64], f16, tag="flatQ")
        flatK = big.tile([64, M * 64], f16, tag="flatK")
        flatV = big.tile([64, M * 64], f32, tag="f32scr")
        for b in range(NBATCH):
            gens = [chain_and_series(initQ, f16, "q", flatQ, b),
                    chain_and_series(initK, f16, "k", flatK, b),
                    chain_and_series(initV, f32, "v", flatV, b)]
            while gens:
                gens = [g for g in gens if next(g, StopIteration) is not StopIteration]

        # ---------------- attention ----------------
        partQ = work.tile([64, M], f32, tag="partQ")
        partK = work.tile([64, M], f32, tag="partK")
        for flat_t, part_t in ((flatQ, partQ), (flatK, partK)):
            sq = big.tile([64, M * 64], f32, tag="VF")
            nc.vector.tensor_mul(sq, flat_t, flat_t)
            nc.vector.tensor_reduce(
                out=part_t, in_=sq.rearrange("p (i c) -> p i c", c=64),
                axis=AX.X, op=OP.add)
        ps_qn = ps_small.tile([1, 64], mybir.dt.float32, tag="small")
        nc.tensor.matmul(ps_qn, ones_col, partQ, start=True, stop=True)
        qn_row = work.tile([1, 64], f32, tag="qnrow_sb")
        nc.vector.tensor_copy(out=qn_row, in_=ps_qn)
        ps_kn = ps_small.tile([64, 1], mybir.dt.float32, tag="small")
        nc.tensor.matmul(ps_kn, partK, ones_col, start=True, stop=True)
        kn_col = work.tile([64, 1], f32, tag="kncol_sb")
        nc.vector.tensor_copy(out=kn_col, in_=ps_kn)
        ps_qrep = ps_small.tile([64, 64], mybir.dt.float32, tag="small")
        nc.tensor.matmul(ps_qrep, ones_row, qn_row, start=True, stop=True)
        qrep = work.tile([64, 64], f32, tag="qrep_sb")
        nc.vector.tensor_copy(out=qrep, in_=ps_qrep)

        ps_cross = ps_small.tile([64, 64], mybir.dt.float32, tag="small")
        fQ3 = flatQ.rearrange("p (i c) -> p c i", c=64)
        fK3 = flatK.rearrange("p (i c) -> p c i", c=64)
        for c in range(64):
            nc.tensor.matmul(ps_cross, fK3[:, c, :], fQ3[:, c, :],
                             start=(c == 0), stop=(c == 63))
        cross_sb = work.tile([64, 64], f32, tag="cross_sb")
        nc.vector.tensor_copy(out=cross_sb, in_=ps_cross)
        Et = work.tile([64, 64], f32, tag="Et")
        nc.vector.scalar_tensor_tensor(out=Et, in0=cross_sb, scalar=-2.0,
                                       in1=qrep, op0=OP.mult, op1=OP.add)
        nc.vector.tensor_scalar(out=Et, in0=Et, scalar1=kn_col, scalar2=0.0,
                                op0=OP.add, op1=OP.max)
        lnE = work.tile([64, 64], f32, tag="lnE")
        nc.scalar.activation(out=lnE, in_=Et, func=ACT.Ln,
                             bias=bias_ln, scale=1.0)
        ln1 = work.tile([64, 64], f32, tag="ln1")
        nc.vector.tensor_scalar_add(ln1, lnE, 1.0)
        sc = work.tile([64, 64], f32, tag="sc")
        nc.vector.reciprocal(out=sc, in_=ln1)
        expS = work.tile([64, 64], f16, tag="expS")
        nc.scalar.activation(out=expS, in_=sc, func=ACT.Exp, bias=0.0, scale=1.0)
        ps_cs = ps_small.tile([64, 1], mybir.dt.float32, tag="small")
        nc.tensor.matmul(ps_cs, expS, ones_col_h, start=True, stop=True)
        inv = work.tile([64, 1], f32, tag="inv")
        nc.vector.reciprocal(out=inv, in_=ps_cs)

        VF = big.tile([64, M * 64], f32, tag="VF")
        VF3 = VF.rearrange("p (r c) -> p r c", c=64)
        for r in range(64):
            nc.gpsimd.dma_start(
                out=VF3[:, r:r + 1, :],
                in_=flatV[r:r + 1, :].rearrange("p (i c) -> p i c", c=64))
        expS32 = work.tile([64, 64], f32, tag="expS32")
        nc.vector.tensor_copy(out=expS32, in_=expS)
        M2 = big.tile([64, M * 64], f32, tag="f32scr")
        for ch in range(8):
            ps_m2 = ps_small.tile([64, 512], mybir.dt.float32, tag="small")
            nc.tensor.matmul(ps_m2, expS32, VF[:, ts(ch, 512)], start=True, stop=True)
            nc.vector.tensor_scalar_mul(M2[:, ts(ch, 512)], ps_m2, inv)

        S1M = big.tile([128, NP_PAIR * 64], f32, tag="scr8c")
        for j in range(M):
            pr, h = j // 2, j % 2
            nc.gpsimd.dma_start(
                out=S1M[h * 64:h * 64 + 64, ts(pr, 64)].rearrange("p (o c) -> p o c", o=1),
                in_=M2[j:j + 1, :].rearrange("p (r c) -> p r c", c=64))

        # ---------------- exp: scaling-squaring ----------------
        outS1 = big.tile([128, NP_PAIR * 64], f32, tag="outS1")
        for b in range(NBATCH):
            cs = ds(b * WB, WB)
            X = S1M[:, cs]
            H = chain.tile([128, WB], f32, tag="expH")
            nc.vector.scalar_tensor_tensor(
                out=H, in0=X, scalar=1.0 / EXP_DEG, in1=IW["32"],
                op0=OP.mult, op1=OP.add)
            for k in range(EXP_DEG - 1, 0, -1):
                psx = ps_mid.tile([128, WB], mybir.dt.float32, tag="ps2")
                mm_pairs(psx, X, H)
                H2 = chain.tile([128, WB], f32, tag="expH")
                nc.vector.tensor_scalar_mul(H2, psx, 1.0 / k)
                nc.vector.tensor_add(H2, H2, IW["32"])
                H = H2
            for sq in range(EXP_SQ):
                psx = ps_mid.tile([128, WB], mybir.dt.float32, tag="ps2")
                mm_pairs(psx, H, H)
                if sq < EXP_SQ - 1:
                    H2 = chain.tile([128, WB], f32, tag="expH")
                    nc.vector.tensor_copy(out=H2, in_=psx)
                    H = H2
                else:
                    nc.vector.tensor_scalar_mul(outS1[:, cs], psx, C_NORM)

        o3 = out_ap[s].rearrange("(pr two) r c -> two r pr c", two=2)
        nc.sync.dma_start(
            out=o3[0], in_=outS1[0:64, :].rearrange("p (pr c) -> p pr c", c=64))
        nc.sync.dma_start(
            out=o3[1], in_=outS1[64:128, :].rearrange("p (pr c) -> p pr c", c=64))


def build(nsamp=NSAMP, num_devices=NCORES):
    import concourse.bacc as bacc
    import concourse.mybir as mybir
    import concourse.tile as tile

    nc = bacc.Bacc("TRN2", target_bir_lowering=False, debug=False,
                   num_devices=num_devices)
    f32 = mybir.dt.float32
    x_ap = nc.dram_tensor("x", [nsamp, M, DIN, DIN], f32, kind="ExternalInput").ap()
    wq = nc.dram_tensor("wq", [DIN, DOUT], f32, kind="ExternalInput").ap()
    wk = nc.dram_tensor("wk", [DIN, DOUT], f32, kind="ExternalInput").ap()
    wv = nc.dram_tensor("wv", [DIN, DOUT], f32, kind="ExternalInput").ap()
    out = nc.dram_tensor("out", [nsamp, M, DOUT, DOUT], f32, kind="ExternalOutput").ap()

    with tile.TileContext(nc) as tc, ExitStack() as ctx:
        emit_kernel(nc, tc, ctx, x_ap, wq, wk, wv, out, nsamp=nsamp, taps={})
    nc.compile()
    return nc


_CACHED = {}


def _get_nc(nsamp):
    from concourse.bass_interp import get_hw_module
    if nsamp not in _CACHED:
        nc = build(nsamp=nsamp)
        nc.m = get_hw_module(nc.m)
        _CACHED[nsamp] = nc
    return _CACHED[nsamp]


def kernel(x, Wq, Wk, Wv):
    from concourse.bass_utils import run_bass_kernel_spmd

    bs = x.shape[0]
    nsamp = bs // NCORES
    nc = _get_nc(nsamp)
    in_maps = []
    for c in range(NCORES):
        in_maps.append({
            "x": np.ascontiguousarray(x[c * nsamp:(c + 1) * nsamp], dtype=np.float32),
            "wq": np.ascontiguousarray(Wq, dtype=np.float32),
            "wk": np.ascontiguousarray(Wk, dtype=np.float32),
            "wv": np.ascontiguousarray(Wv, dtype=np.float32),
        })
    res = run_bass_kernel_spmd(nc, in_maps, list(range(NCORES)))
    outs = [res.results[c]["out"] for c in range(NCORES)]
    full = np.concatenate(outs, axis=0)
    return full.reshape(bs * M, DOUT, DOUT).astype(np.float32)



# revision 14
# speedup vs baseline: 1.3166x; 1.3166x over previous
"""Trainium2 Bass kernel for nn_AttentionManifold (SPD manifold attention).

For each of bs*m=2048 SPD matrices X (100x100): Q/K/V = W^T X W (64x64),
logQ/K/V = matrix log, log-Euclidean attention, mixed = prob-weighted sum
of logV, out = matrix exp(mixed).

Matrix log: Newton-Schulz coupled sqrt chain, L=3 levels, with
first-order level corrections  log A = 2^L log Y_L - sum_l 2^l log W_l,
log W ~= -(I - W);  series log(Y) via deg-8/12 Paterson-Stockmeyer.

Chain scheme '2s_approx' (emulator-validated): exact-transpose pairs
(Y, Yt, Z, Zt) in fp16, P = aI + bW only (no Pt):
    W    = {lhsT=Zt_h, rhs=Y_h}              (per matrix)
    P    = aI + b psW                        (stt)
    Y'   = {lhsT=Yt_h, rhs=P_h}              = Y P
    [Yt'|Z'] = {lhsT=bd(P), rhs=[Yt|Z]}      = [P^T Yt | P^T Z]  (packed)
    Zt'  = {lhsT=Z_h, rhs=P_h}               = (P^T Z)^T bitwise
Every W is a congruence of the SPD input => fp16-stable.

exp via scaling-squaring (deg-4 Horner, 4 squarings).
Sharding: pure data parallelism, bs=32 -> 4 samples per NeuronCore.
"""
import numpy as np
from contextlib import ExitStack

C_NORM = 16.0
BS, M, DIN, DOUT = 32, 64, 100, 64
NCORES = 8
NSAMP = BS // NCORES
NP_PAIR = M // 2          # 32 pairs per sample
PB = 4                    # pairs per chain batch
NBATCH = NP_PAIR // PB    # 8

SCHED_V = [
    [(5.005419879, -6.230249553), (2.163837188, -0.504783185),
     (2.177777778, -0.511194856), (2.163837188, -0.501880648),
     (2.163837188, -0.502320999), (1.17846369, -0.211941406),
     (1.507505828, -0.504295313)],
    [(4.324444444, -4.005424891), (2.163837188, -0.502241042),
     (1.24171808, -0.238032231), (1.507505828, -0.499772111)],
    [(4.009075369, -3.220580001), (1.077909748, -0.173454032),
     (1.49854139, -0.498620446)],
]
SCHED_QK = [
    [(8.965874126, -13.460097634), (2.380408822, -0.250737931),
     (2.380408822, -0.250532192), (2.380408822, -0.250326648),
     (0.861964497, -0.071654452), (1.542284382, -0.519941516)],
    [(7.758850039, -8.666077201), (0.987610378, -0.093162713),
     (1.645967366, -0.5826622), (1.507505828, -0.502426376)],
    [(6.551825952, -5.22018671), (0.65339645, -0.038866921)],
]
DEG_QK = 8
DEG_V = 12
EXP_DEG = 4
EXP_SQ = 4
L = 3
WBS = PB * 64             # 256
SBW = PB * 256            # 1024


def emit_kernel(nc, tc, ctx, x_ap, wq_ap, wk_ap, wv_ap, out_ap, nsamp=NSAMP,
                taps=None):
    def tap(name, t):
        if taps is not None and name in taps:
            nc.sync.dma_start(out=taps[name], in_=t)

    import concourse.mybir as mybir
    from concourse.bass import ds, ts
    from concourse.masks import make_identity

    f32 = mybir.dt.float32
    f32r = mybir.dt.float32r
    f16 = mybir.dt.float16
    AX = mybir.AxisListType
    OP = mybir.AluOpType
    ACT = mybir.ActivationFunctionType

    # ---- engine rotation helpers ----
    _rrc = [0]
    _rrs = [0]

    from concourse.bass import MemorySpace as _MS

    def _psum(*aps):
        return any(a.space == _MS.PSUM for a in aps)

    def rr_copy(out, in_, scale=None):
        pool = ((nc.vector, nc.scalar) if _psum(out, in_)
                else (nc.vector, nc.scalar, nc.gpsimd))
        e = pool[_rrc[0] % len(pool)]
        _rrc[0] += 1
        if e is nc.scalar:
            nc.scalar.activation(out=out, in_=in_, func=ACT.Copy, bias=0.0,
                                 scale=1.0 if scale is None else float(scale))
        elif scale is None:
            e.tensor_copy(out=out, in_=in_)
        else:
            e.tensor_scalar_mul(out, in_, float(scale))

    def rr_stt(out, in0, scalar, in1):
        nc.vector.scalar_tensor_tensor(out=out, in0=in0, scalar=float(scalar),
                                       in1=in1, op0=OP.mult, op1=OP.add)

    # ---------------- pools ----------------
    const = ctx.enter_context(tc.tile_pool(name="const", bufs=1))
    work = ctx.enter_context(tc.tile_pool(name="work", bufs=2))
    big = ctx.enter_context(tc.tile_pool(name="big", bufs=1))
    chain = ctx.enter_context(tc.tile_pool(name="chain", bufs=2))
    ps_w = ctx.enter_context(tc.tile_pool(name="ps_w", bufs=1, space="PSUM"))
    ps_bc = ctx.enter_context(tc.tile_pool(name="ps_bc", bufs=2, space="PSUM"))
    ps_a = ctx.enter_context(tc.tile_pool(name="ps_a", bufs=1, space="PSUM"))
    ps_b = ctx.enter_context(tc.tile_pool(name="ps_b", bufs=1, space="PSUM"))

    # ---------------- constants ----------------
    W3f = const.tile([DIN, 3 * DOUT], f32)
    nc.sync.dma_start(out=W3f[:, 0:DOUT], in_=wq_ap)
    nc.sync.dma_start(out=W3f[:, DOUT:2 * DOUT], in_=wk_ap)
    nc.sync.dma_start(out=W3f[:, 2 * DOUT:3 * DOUT], in_=wv_ap)
    W3r = const.tile([DIN, 256], f32r)
    nc.vector.memset(W3r.bitcast(f32), 0.0)
    nc.vector.tensor_copy(out=W3r[:, 0:192], in_=W3f)
    WQKh = const.tile([DIN, 128], f16)        # f16 weights for Q/K stage-2
    nc.vector.tensor_copy(out=WQKh, in_=W3f[:, 0:128])

    IREP = const.tile([128, 64], f16)
    make_identity(nc, IREP[0:64, :])
    make_identity(nc, IREP[64:128, :])

    aI = {}

    def get_aI(val):
        val = float(val)
        if val not in aI:
            t = const.tile([128, WBS], f16, tag=f"aI{len(aI)}",
                           name=f"aI{len(aI)}")
            for p in range(PB):
                nc.vector.tensor_scalar_mul(t[:, ts(p, 64)], IREP, val)
            aI[val] = t
        return aI[val]

    for lv in SCHED_QK + SCHED_V:
        for a, b in lv:
            get_aI(a)
    for c0 in (1.0, 0.25, 0.125):
        get_aI(c0)
    I7f = const.tile([128, WBS], f32)
    for p in range(PB):
        nc.vector.tensor_scalar_mul(I7f[:, ts(p, 64)], IREP, float(2 ** L - 1))
    cI_mm = {}

    def get_cI(val):
        val = float(val)
        if val not in cI_mm:
            t = const.tile([128, 64], f16, tag=f"cImm{len(cI_mm)}",
                           name=f"cImm{len(cI_mm)}")
            nc.vector.tensor_scalar_mul(t, IREP, val)
            cI_mm[val] = t
        return cI_mm[val]

    for v in (1.0, 0.5, 1.0 / 3.0, 2.0, 3.0):
        get_cI(v)
    for lv in SCHED_QK + SCHED_V:
        for a, b in lv:
            get_cI(a / b)
    for v in (0.25, 0.2, 1.0 / 6, 1.0 / 7, 0.125, 1.0 / 9, 0.1,
              1.0 / 11, 1.0 / 12):
        get_cI(v)

    ones_col = const.tile([64, 1], f32)
    nc.vector.memset(ones_col, 1.0)
    ones_col_h = const.tile([64, 1], f16)
    nc.vector.memset(ones_col_h, 16.0)    # folds 1/16 exp prescale into inv
    ones_row = const.tile([1, 64], f32)
    nc.vector.memset(ones_row, 1.0)
    bias_ln = const.tile([64, 1], f32)
    nc.vector.memset(bias_ln, 1.0 + 64e-6)

    # DRAM scratch for partition-moving transposes (DRAM APs unconstrained)
    scrV = nc.dram_tensor("scrV", [64, M * 64], f16, kind="Internal").ap()
    scrM = nc.dram_tensor("scrM", [64, M * 64], f16, kind="Internal").ap()

    # bdP buffers zero-initialized once; only diag slots rewritten after
    for cn in ("q", "k", "v"):
        for i in (0, 1):
            t = chain.tile([128, PB * 128], f16, tag=f"bdP{cn}",
                           name=f"bdPz{cn}{i}")
            nc.vector.memset(t, 0.0)

    def r3(t):
        """[p, (n c)] -> [p, n, 64] view"""
        return t.rearrange("p (n c) -> p n c", c=64)

    def slot(S, f):
        """S [128, (PB,4,64)] -> slot view [128, PB, 64]"""
        return S.rearrange("p (n four c) -> p n four c", four=4, c=64)[:, :, f, :]

    # =====================================================================
    def chain_gen(cn, sched, deg, init_t, b, flat_t):
        """One chain batch: NS chain + corrections + series -> flat_t."""
        ib = r3(init_t[:, ds(b * PB * 64, PB * 64)])     # [128, PB, 64]
        S_cur = None
        Y = Yt = Z = Zt = None       # [128, PB, 64] views
        adj = False
        ACC = None

        for l in range(L):
            steps = sched[l]
            for j, (a, bc) in enumerate(steps):
                Pd = chain.tile([128, WBS], f16, tag=f"Pd{cn}", name=f"Pd{cn}")
                bdP = chain.tile([128, PB * 128], f16, tag=f"bdP{cn}",
                                 name=f"bdPu{cn}")
                Pd3 = r3(Pd)
                bd128 = bdP.rearrange("p (n c) -> p n c", c=128)
                aIt = get_aI(a)
                if j == 0:
                    src = ib if l == 0 else Y
                    rr_stt(Pd3, src, bc, r3(aIt))
                    rr_stt(bd128[0:64, :, 0:64], src[0:64], bc,
                           r3(aIt)[0:64])
                    rr_stt(bd128[64:128, :, 64:128], src[64:128], bc,
                           r3(aIt)[64:128])
                    yield
                else:
                    psW = ps_w.tile([128, WBS], f32, tag="psW", name="psW")
                    psW3 = r3(psW)
                    abI = get_cI(a / bc)
                    Ipat = get_aI(1.0)
                    for h in (0, 1):
                        hs = slice(h * 64, h * 64 + 64)
                        nc.tensor.matmul(psW[hs, :], abI[hs, :], Ipat[hs, :],
                                         start=True, stop=False)
                    for p in range(PB):
                        for h in (0, 1):
                            hs = slice(h * 64, h * 64 + 64)
                            nc.tensor.matmul(psW3[hs, p], Zt[hs, p], Y[hs, p],
                                             start=False,
                                             stop=(p == PB - 1),
                                             skip_group_check=True)
                    yield
                    rr_copy(Pd, psW, scale=bc)
                    rr_copy(bd128[0:64, :, 0:64], psW3[0:64], scale=bc)
                    rr_copy(bd128[64:128, :, 64:128], psW3[64:128], scale=bc)
                    yield
                # ---- updates ----
                psBC = ps_bc.tile([128, SBW], f32, tag="psBC", name="psBC")
                ps4 = psBC.rearrange("p (n four c) -> p n four c", four=4, c=64)
                YtP = ib if (l == 0 and j == 0) else Yt
                for p in range(PB):
                    for h in (0, 1):
                        hs = slice(h * 64, h * 64 + 64)
                        # Y' = Yt^T P
                        nc.tensor.matmul(ps4[hs, p, 0, :], YtP[hs, p],
                                         Pd3[hs, p], start=True, stop=True)
                    bdfull = bdP[:, p * 128:p * 128 + 128]
                    if adj:
                        # [Yt'|Z'] packed: rhs = S[Yt|Z] adjacent
                        nc.tensor.matmul(
                            psBC[:, p * 256 + 64:p * 256 + 192],
                            bdfull, S_cur[:, p * 256 + 64:p * 256 + 192],
                            start=True, stop=True)
                    else:
                        nc.tensor.matmul(ps4[:, p, 1, :], bdfull,
                                         YtP[:, p], start=True, stop=True)
                        if j > 0:
                            nc.tensor.matmul(ps4[:, p, 2, :], bdfull,
                                             Z[:, p], start=True, stop=True)
                    if j > 0:
                        for h in (0, 1):
                            hs = slice(h * 64, h * 64 + 64)
                            # Zt' = Z^T P
                            nc.tensor.matmul(ps4[hs, p, 3, :], Z[hs, p],
                                             Pd3[hs, p], start=True, stop=True)
                yield
                S_new = chain.tile([128, SBW], f16, tag=f"S{cn}", name=f"S{cn}")
                s4 = S_new.rearrange("p (n four c) -> p n four c", four=4, c=64)
                if j == 0:
                    rr_copy(s4[:, :, 0:2, :], ps4[:, :, 0:2, :])
                    Z = Pd3
                    Zt = Pd3
                    adj = False
                else:
                    rr_copy(S_new, psBC)
                    Z = slot(S_new, 2)
                    Zt = slot(S_new, 3)
                    adj = True
                Y = slot(S_new, 0)
                Yt = slot(S_new, 1)
                S_cur = S_new
                yield
            # ---- level end correction: psWe = Zt^T Y + Y^T Zt ----
            psWe = ps_w.tile([128, WBS], f32, tag="psW", name="psWe")
            pw3 = r3(psWe)
            for p in range(PB):
                for h in (0, 1):
                    hs = slice(h * 64, h * 64 + 64)
                    nc.tensor.matmul(pw3[hs, p], Zt[hs, p], Y[hs, p],
                                     start=True, stop=False)
                    nc.tensor.matmul(pw3[hs, p], Y[hs, p], Zt[hs, p],
                                     start=False, stop=True)
            yield
            if l == 0:
                ACC = chain.tile([128, WBS], f32, tag=f"acc{cn}",
                                 name=f"acc{cn}")
                rr_stt(ACC, psWe, -0.5, I7f)
            else:
                rr_stt(ACC, psWe, -float(2 ** l) / 2.0, ACC)
            yield
        # ================= series =================
        E = chain.tile([128, WBS], f16, tag=f"E{cn}", name=f"E{cn}")
        rr_stt(r3(E), Y, -1.0, r3(get_aI(1.0)))
        yield
        powers = {1: E}
        for k, rt in ((2, 1), (3, 2), (4, 3)):
            psE = ps_a.tile([128, WBS], f32, tag="scrA", name="psE")
            pe3 = r3(psE)
            e1 = r3(powers[1])
            ert = r3(powers[rt])
            for p in range(PB):
                for h in (0, 1):
                    hs = slice(h * 64, h * 64 + 64)
                    nc.tensor.matmul(pe3[hs, p], e1[hs, p], ert[hs, p],
                                     start=True, stop=True)
            Ek = chain.tile([128, WBS], f16, tag=f"E{k}{cn}", name=f"E{k}{cn}")
            rr_copy(Ek, psE)
            powers[k] = Ek
            yield
        E2, E3, E4 = powers[2], powers[3], powers[4]

        def combo(coefs, dst_tag):
            """PE-accumulated c0 I + c1 E + c2 E2 + c3 E3 + c4 E4 -> f16."""
            psC = ps_a.tile([128, WBS], f32, tag="scrA", name="psC")
            ops = [(get_cI(coefs[0]), get_aI(1.0))] + [
                (get_cI(cv), pw) for cv, pw in
                zip(coefs[1:], (E, E2, E3, E4)) if cv]
            for i, (lh, rh) in enumerate(ops):
                for h in (0, 1):
                    hs = slice(h * 64, h * 64 + 64)
                    nc.tensor.matmul(psC[hs, :], lh[hs, :], rh[hs, :],
                                     start=(i == 0),
                                     stop=(i == len(ops) - 1),
                                     skip_group_check=(i > 0))
            Ct = chain.tile([128, WBS], f16, tag=dst_tag, name=dst_tag)
            rr_copy(Ct, psC)
            return Ct

        C = combo((0.25, 0.2, 1.0 / 6, 1.0 / 7, 0.125), f"C{cn}")
        yield
        if deg == 12:
            C2 = combo((0.125, 1.0 / 9, 0.1, 1.0 / 11, 1.0 / 12), f"C2{cn}")
            psH = ps_a.tile([128, WBS], f32, tag="scrA", name="psH")
            ph3, e43, c23 = r3(psH), r3(E4), r3(C2)
            for p in range(PB):
                for h in (0, 1):
                    hs = slice(h * 64, h * 64 + 64)
                    nc.tensor.matmul(ph3[hs, p], e43[hs, p], c23[hs, p],
                                     start=True, stop=True)
            yield
            CH = chain.tile([128, WBS], f16, tag=f"C2{cn}", name=f"CH{cn}")
            nc.vector.tensor_tensor(out=CH, in0=psH, in1=C, op=OP.add)
            C = CH
            yield
        # psB0 = 1*E + E4@C + (1/2)E2 + (1/3)E3   (accumulated group)
        psB0 = ps_a.tile([128, WBS], f32, tag="scrA", name="psB0")
        e43, c3 = r3(E4), r3(C)
        for h in (0, 1):
            hs = slice(h * 64, h * 64 + 64)
            nc.tensor.matmul(psB0[hs, :], get_cI(1.0)[hs, :], E[hs, :],
                             start=True, stop=False)
        for p in range(PB):
            for h in (0, 1):
                hs = slice(h * 64, h * 64 + 64)
                nc.tensor.matmul(r3(psB0)[hs, p], e43[hs, p], c3[hs, p],
                                 start=False, stop=False,
                                 skip_group_check=True)
        for h in (0, 1):
            hs = slice(h * 64, h * 64 + 64)
            nc.tensor.matmul(psB0[hs, :], get_cI(0.5)[hs, :], E2[hs, :],
                             start=False, stop=False, skip_group_check=True)
            nc.tensor.matmul(psB0[hs, :], get_cI(1.0 / 3.0)[hs, :], E3[hs, :],
                             start=False, stop=True, skip_group_check=True)
        yield
        # LS = -2^L psB0 + ACC -> flat (strided, per h)
        fl3 = flat_t.rearrange("p (pr two c) -> p pr two c", two=2, c=64)
        acc3 = r3(ACC)
        for h in (0, 1):
            hs = slice(h * 64, h * 64 + 64)
            rr_stt(fl3[:, ds(b * PB, PB), h, :], r3(psB0)[hs], -float(2 ** L),
                   acc3[hs])
        yield

    # ======================= per-sample pipeline =========================
    for s in range(nsamp):
        initQ = work.tile([128, NP_PAIR * 64], f16, tag="initQ", name="initQ")
        initK = work.tile([128, NP_PAIR * 64], f16, tag="initK", name="initK")
        initV = work.tile([128, NP_PAIR * 64], f16, tag="initV", name="initV")

        # ---------------- congruence ----------------
        for g in range(8):          # 8 matrices per group
            if g % 2 == 0:
                xbuf = work.tile([DIN, 16 * DIN], f32r, tag="xbuf", name="xbuf")
                nc.gpsimd.dma_start(
                    out=xbuf.rearrange("p (i c) -> p i c", c=DIN),
                    in_=x_ap[s, ds(g * 8, 16)].rearrange("i p c -> p i c"))
            pqks = []
            pvs = []
            for r in range(2):      # 2 rounds x 4 matrices
                ps1 = ps_b.tile([DIN, 4 * 256], f32, tag="scrB", name="ps1")
                for mi in range(4):
                    mg = (g % 2) * 8 + r * 4 + mi
                    nc.tensor.matmul(ps1[:, ts(mi, 256)],
                                     xbuf[:, ts(mg, DIN)], W3r,
                                     start=True, stop=True)
                pqk = work.tile([DIN, 4 * 128], f16, tag="pqk", name="pqk")
                pv = work.tile([DIN, 4 * 64], f32, tag="pv", name="pv")
                rr_copy(pqk.rearrange("p (n c) -> p n c", c=128),
                        ps1.rearrange("p (n c) -> p n c", c=256)[:, :, 0:128])
                rr_copy(pv.rearrange("p (n c) -> p n c", c=64),
                        ps1.rearrange("p (n c) -> p n c", c=256)[:, :, 128:192])
                pqks.append(pqk)
                pvs.append(pv)
            for wi, init_t in ((0, initQ), (1, initK), (2, initV)):
                psI = ps_w.tile([128, WBS], f32, tag="psW", name="psI")
                for m in range(8):
                    r, mi = m // 4, m % 4
                    pr, h = m // 2, m % 2
                    hs = slice(h * 64, h * 64 + 64)
                    if wi < 2:
                        rhs = pqks[r][:, mi * 128 + wi * 64:
                                      mi * 128 + wi * 64 + 64]
                    else:
                        rhs = pvs[r][:, ts(mi, 64)]
                    lhsW = (WQKh[:, ts(wi, 64)] if wi < 2
                            else W3f[:, ts(2, 64)])
                    nc.tensor.matmul(psI[hs, ts(pr, 64)], lhsW, rhs,
                                     start=True, stop=True)
                rr_copy(init_t[:, ds(g * 4 * 64, WBS)], psI, scale=1.0 / C_NORM)

        if s == 0:
            tap("initQ", initQ)
            tap("initK", initK)
            tap("initV", initV)
        # ---------------- chains ----------------
        flatQ = big.tile([64, M * 64], f16, tag="flatQ", name="flatQ")
        flatK = big.tile([64, M * 64], f16, tag="flatK", name="flatK")
        flatV = big.tile([64, M * 64], f16, tag="flatV", name="flatV")
        for b in range(NBATCH):
            gens = [chain_gen("q", SCHED_QK, DEG_QK, initQ, b, flatQ),
                    chain_gen("k", SCHED_QK, DEG_QK, initK, b, flatK),
                    chain_gen("v", SCHED_V, DEG_V, initV, b, flatV)]
            while gens:
                gens = [g for g in gens
                        if next(g, StopIteration) is not StopIteration]

        if s == 0:
            tap("flatQ", flatQ)
            tap("flatK", flatK)
            tap("flatV", flatV)
        # ---------------- attention ----------------
        partQ = work.tile([64, M], f32, tag="partQ", name="partQ")
        partK = work.tile([64, M], f32, tag="partK", name="partK")
        for flat_t, part_t in ((flatQ, partQ), (flatK, partK)):
            sq = big.tile([64, M * 64], f16, tag="sqscr", name="sqscr")
            nc.vector.tensor_mul(sq, flat_t, flat_t)
            nc.vector.tensor_reduce(
                out=part_t, in_=sq.rearrange("p (i c) -> p i c", c=64),
                axis=AX.X, op=OP.add)
        ps_qn = ps_a.tile([1, 64], f32, tag="scrA", name="ps_qn")
        nc.tensor.matmul(ps_qn, ones_col, partQ, start=True, stop=True)
        qn_row = work.tile([1, 64], f32, tag="qnrow", name="qnrow")
        nc.vector.tensor_copy(out=qn_row, in_=ps_qn)
        ps_kn = ps_a.tile([64, 1], f32, tag="scrA", name="ps_kn")
        nc.tensor.matmul(ps_kn, partK, ones_col, start=True, stop=True)
        kn_col = work.tile([64, 1], f32, tag="kncol", name="kncol")
        nc.vector.tensor_copy(out=kn_col, in_=ps_kn)
        ps_qrep = ps_a.tile([64, 64], f32, tag="scrA", name="ps_qrep")
        nc.tensor.matmul(ps_qrep, ones_row, qn_row, start=True, stop=True)
        qrep = work.tile([64, 64], f32, tag="qrep", name="qrep")
        nc.vector.tensor_copy(out=qrep, in_=ps_qrep)

        ps_cross = ps_a.tile([64, 64], f32, tag="scrA", name="ps_cross")
        fQ3 = flatQ.rearrange("p (i c) -> p c i", c=64)
        fK3 = flatK.rearrange("p (i c) -> p c i", c=64)
        for c in range(64):
            nc.tensor.matmul(ps_cross, fK3[:, c, :], fQ3[:, c, :],
                             start=(c == 0), stop=(c == 63))
        Et = work.tile([64, 64], f32, tag="Et", name="Et")
        nc.vector.scalar_tensor_tensor(out=Et, in0=ps_cross, scalar=-2.0,
                                       in1=qrep, op0=OP.mult, op1=OP.add)
        nc.vector.tensor_scalar(out=Et, in0=Et, scalar1=kn_col, scalar2=0.0,
                                op0=OP.add, op1=OP.max)
        lnE = work.tile([64, 64], f32, tag="lnE", name="lnE")
        nc.scalar.activation(out=lnE, in_=Et, func=ACT.Ln,
                             bias=bias_ln, scale=1.0)
        ln1 = work.tile([64, 64], f32, tag="ln1", name="ln1")
        nc.vector.tensor_scalar_add(ln1, lnE, 1.0)
        sc = work.tile([64, 64], f32, tag="sc", name="sc")
        nc.vector.reciprocal(out=sc, in_=ln1)
        expS = work.tile([64, 64], f16, tag="expS", name="expS")
        nc.scalar.activation(out=expS, in_=sc, func=ACT.Exp, bias=0.0,
                             scale=1.0)
        ps_cs = ps_a.tile([64, 1], f32, tag="scrA", name="ps_cs")
        nc.tensor.matmul(ps_cs, expS, ones_col_h, start=True, stop=True)
        inv = work.tile([64, 1], f32, tag="inv", name="inv")
        nc.vector.reciprocal(out=inv, in_=ps_cs)

        # VF: flatV [p, (i c)] -> VF [i, (p c)] via DRAM roundtrip
        VF = big.tile([64, M * 64], f16, tag="VF", name="VF")
        nc.sync.dma_start(out=scrV, in_=flatV)
        nc.sync.dma_start(
            out=VF.rearrange("i (p c) -> i p c", c=64),
            in_=scrV.rearrange("p (i c) -> i p c", c=64))
        if s == 0:
            tap("VF", VF)
        # mixing: M2[j, (p c)] = sum_i expS[i, j] VF[i, (p c)] * inv[j]
        M2 = big.tile([64, M * 64], f16, tag="M2", name="M2")
        for ch in range(4):
            ps_m = ps_b.tile([64, 1024], f32, tag="scrB", name="ps_m")
            nc.tensor.matmul(ps_m[:, 0:512], expS, VF[:, ds(ch * 1024, 512)],
                             start=True, stop=True)
            nc.tensor.matmul(ps_m[:, 512:1024], expS,
                             VF[:, ds(ch * 1024 + 512, 512)],
                             start=True, stop=True)
            nc.vector.tensor_scalar_mul(M2[:, ds(ch * 1024, 1024)], ps_m, inv)
        # S1M scatter: M2 [j=(pr h), (p c)] -> S1M [(h p), (pr c)] via DRAM
        S1M = big.tile([128, NP_PAIR * 64], f16, tag="S1M", name="S1M")
        nc.sync.dma_start(out=scrM, in_=M2)
        for h in (0, 1):
            nc.sync.dma_start(
                out=S1M[h * 64:(h + 1) * 64, :].rearrange(
                    "p (pr c) -> p pr c", c=64),
                in_=scrM.rearrange("(pr two) (p c) -> two p pr c",
                                   two=2, c=64)[h])

        if s == 0:
            tap("M2", M2)
            tap("S1M", S1M)
            tap("expS", expS)
        # ---------------- exp ----------------
        outS1 = big.tile([128, NP_PAIR * 64], f32, tag="outS1", name="outS1")
        for b in range(NBATCH):
            Xs = S1M[:, ds(b * WBS, WBS)]
            X3 = r3(Xs)
            H = chain.tile([128, WBS], f16, tag="expH", name="expH")
            rr_stt(H, Xs, 1.0 / EXP_DEG, get_aI(1.0))
            for k in range(EXP_DEG - 1, 0, -1):
                psx = ps_a.tile([128, WBS], f32, tag="scrA", name="psx")
                px3, h3 = r3(psx), r3(H)
                for hh in (0, 1):
                    hs = slice(hh * 64, hh * 64 + 64)
                    nc.tensor.matmul(psx[hs, :], get_cI(float(k))[hs, :],
                                     get_aI(1.0)[hs, :],
                                     start=True, stop=False)
                for p in range(PB):
                    for hh in (0, 1):
                        hs = slice(hh * 64, hh * 64 + 64)
                        nc.tensor.matmul(px3[hs, p], X3[hs, p], h3[hs, p],
                                         start=False, stop=(p == PB - 1),
                                         skip_group_check=True)
                H2 = chain.tile([128, WBS], f16, tag="expH", name="expH2")
                rr_copy(H2, psx, scale=1.0 / k)
                H = H2
            for sq_i in range(EXP_SQ):
                psx = ps_a.tile([128, WBS], f32, tag="scrA", name="psx2")
                px3, h3 = r3(psx), r3(H)
                for p in range(PB):
                    for hh in (0, 1):
                        hs = slice(hh * 64, hh * 64 + 64)
                        nc.tensor.matmul(px3[hs, p], h3[hs, p], h3[hs, p],
                                         start=True, stop=True)
                if sq_i < EXP_SQ - 1:
                    H2 = chain.tile([128, WBS], f16, tag="expH", name="expH3")
                    rr_copy(H2, psx)
                    H = H2
                else:
                    rr_copy(outS1[:, ds(b * WBS, WBS)], psx, scale=C_NORM)

        o3 = out_ap[s].rearrange("(pr two) r c -> two r pr c", two=2)
        nc.sync.dma_start(
            out=o3[0],
            in_=outS1[0:64, :].rearrange("p (pr c) -> p pr c", c=64))
        nc.sync.dma_start(
            out=o3[1],
            in_=outS1[64:128, :].rearrange("p (pr c) -> p pr c", c=64))


def build(nsamp=NSAMP, num_devices=NCORES, debug_taps=False):
    import concourse.bacc as bacc
    import concourse.mybir as mybir
    import concourse.tile as tile

    nc = bacc.Bacc("TRN2", target_bir_lowering=False, debug=False,
                   num_devices=num_devices)
    f32 = mybir.dt.float32
    x_ap = nc.dram_tensor("x", [nsamp, M, DIN, DIN], f32,
                          kind="ExternalInput").ap()
    wq = nc.dram_tensor("wq", [DIN, DOUT], f32, kind="ExternalInput").ap()
    wk = nc.dram_tensor("wk", [DIN, DOUT], f32, kind="ExternalInput").ap()
    wv = nc.dram_tensor("wv", [DIN, DOUT], f32, kind="ExternalInput").ap()
    out = nc.dram_tensor("out", [nsamp, M, DOUT, DOUT], f32,
                         kind="ExternalOutput").ap()

    taps = {}
    if debug_taps:
        for nm, shp, dt_ in (("initQ", [128, 2048], mybir.dt.float16),
                             ("initK", [128, 2048], mybir.dt.float16),
                             ("initV", [128, 2048], mybir.dt.float16),
                             ("flatQ", [64, 4096], mybir.dt.float16),
                             ("flatK", [64, 4096], mybir.dt.float16),
                             ("flatV", [64, 4096], mybir.dt.float16),
                             ("M2", [64, 4096], mybir.dt.float16),
                             ("VF", [64, 4096], mybir.dt.float16),
                             ("S1M", [128, 2048], mybir.dt.float16),
                             ("expS", [64, 64], mybir.dt.float16)):
            taps[nm] = nc.dram_tensor("tap_" + nm, shp, dt_,
                                      kind="ExternalOutput").ap()
    with tile.TileContext(nc) as tc, ExitStack() as ctx:
        emit_kernel(nc, tc, ctx, x_ap, wq, wk, wv, out, nsamp=nsamp,
                    taps=taps if debug_taps else None)
    nc.compile()
    return nc


_CACHED = {}


def _get_nc(nsamp):
    from concourse.bass_interp import get_hw_module
    if nsamp not in _CACHED:
        nc = build(nsamp=nsamp)
        nc.m = get_hw_module(nc.m)
        _CACHED[nsamp] = nc
    return _CACHED[nsamp]


def kernel(x, Wq, Wk, Wv):
    from concourse.bass_utils import run_bass_kernel_spmd

    bs = x.shape[0]
    nsamp = bs // NCORES
    nc = _get_nc(nsamp)
    in_maps = []
    for c in range(NCORES):
        in_maps.append({
            "x": np.ascontiguousarray(x[c * nsamp:(c + 1) * nsamp],
                                      dtype=np.float32),
            "wq": np.ascontiguousarray(Wq, dtype=np.float32),
            "wk": np.ascontiguousarray(Wk, dtype=np.float32),
            "wv": np.ascontiguousarray(Wv, dtype=np.float32),
        })
    res = run_bass_kernel_spmd(nc, in_maps, list(range(NCORES)))
    outs = [res.results[c]["out"] for c in range(NCORES)]
    full = np.concatenate(outs, axis=0)
    return full.reshape(bs * M, DOUT, DOUT).astype(np.float32)


# revision 15
# speedup vs baseline: 1.6501x; 1.2533x over previous
"""Trainium2 Bass kernel for nn_AttentionManifold (SPD manifold attention).

For each of bs*m=2048 SPD matrices X (100x100): Q/K/V = W^T X W (64x64),
logQ/K/V = matrix log, log-Euclidean attention, mixed = prob-weighted sum
of logV, out = matrix exp(mixed).

Matrix log: Newton-Schulz coupled sqrt chain, L=3 levels, with
first-order level corrections  log A = 2^L log Y_L - sum_l 2^l log W_l,
log W ~= -(I - W);  series log(Y) via deg-8/12 Paterson-Stockmeyer.

Chain scheme '2s_approx' (emulator-validated): exact-transpose pairs
(Y, Yt, Z, Zt) in fp16, P = aI + bW only (no Pt):
    W    = {lhsT=Zt_h, rhs=Y_h}              (per matrix)
    P    = aI + b psW                        (stt)
    Y'   = {lhsT=Yt_h, rhs=P_h}              = Y P
    [Yt'|Z'] = {lhsT=bd(P), rhs=[Yt|Z]}      = [P^T Yt | P^T Z]  (packed)
    Zt'  = {lhsT=Z_h, rhs=P_h}               = (P^T Z)^T bitwise
Every W is a congruence of the SPD input => fp16-stable.

exp via scaling-squaring (deg-4 Horner, 4 squarings).
Sharding: pure data parallelism, bs=32 -> 4 samples per NeuronCore.
"""
import numpy as np
from contextlib import ExitStack

C_NORM = 16.0
BS, M, DIN, DOUT = 32, 64, 100, 64
NCORES = 8
NSAMP = BS // NCORES
NP_PAIR = M // 2          # 32 pairs per sample
PB = 4                    # pairs per chain batch
NBATCH = NP_PAIR // PB    # 8

SCHED_V = [
    [(5.005419879, -6.230249553), (2.163837188, -0.504783185),
     (2.177777778, -0.511194856), (2.163837188, -0.501880648),
     (2.163837188, -0.502320999), (1.17846369, -0.211941406),
     (1.507505828, -0.504295313)],
    [(4.324444444, -4.005424891), (2.163837188, -0.502241042),
     (1.24171808, -0.238032231), (1.507505828, -0.499772111)],
    [(4.009075369, -3.220580001), (1.077909748, -0.173454032),
     (1.49854139, -0.498620446)],
]
SCHED_QK = [
    [(8.965874126, -13.460097634), (2.380408822, -0.250737931),
     (2.380408822, -0.250532192), (2.380408822, -0.250326648),
     (0.861964497, -0.071654452), (1.542284382, -0.519941516)],
    [(7.758850039, -8.666077201), (0.987610378, -0.093162713),
     (1.645967366, -0.5826622), (1.507505828, -0.502426376)],
    [(6.551825952, -5.22018671), (0.65339645, -0.038866921)],
]
DEG_QK = 8
DEG_V = 12
EXP_DEG = 4
EXP_SQ = 4
L = 3
WBS = PB * 64             # 256
SBW = PB * 256            # 1024


def emit_kernel(nc, tc, ctx, x_ap, wq_ap, wk_ap, wv_ap, out_ap, nsamp=NSAMP,
                taps=None):
    def tap(name, t):
        if taps is not None and name in taps:
            nc.sync.dma_start(out=taps[name], in_=t)

    import concourse.mybir as mybir
    from concourse.bass import ds, ts
    from concourse.masks import make_identity

    f32 = mybir.dt.float32
    f32r = mybir.dt.float32r
    f16 = mybir.dt.float16
    AX = mybir.AxisListType
    OP = mybir.AluOpType
    ACT = mybir.ActivationFunctionType

    # ---- engine rotation helpers ----
    _rrc = [0]
    _rrs = [0]

    from concourse.bass import MemorySpace as _MS

    def _psum(*aps):
        return any(a.space == _MS.PSUM for a in aps)

    def rr_copy(out, in_, scale=None):
        pool = ((nc.vector, nc.scalar) if _psum(out, in_)
                else (nc.vector, nc.scalar, nc.gpsimd))
        e = pool[_rrc[0] % len(pool)]
        _rrc[0] += 1
        if e is nc.scalar:
            nc.scalar.activation(out=out, in_=in_, func=ACT.Copy, bias=0.0,
                                 scale=1.0 if scale is None else float(scale))
        elif scale is None:
            e.tensor_copy(out=out, in_=in_)
        else:
            e.tensor_scalar_mul(out, in_, float(scale))

    def rr_stt(out, in0, scalar, in1):
        nc.vector.scalar_tensor_tensor(out=out, in0=in0, scalar=float(scalar),
                                       in1=in1, op0=OP.mult, op1=OP.add)

    # ---------------- pools ----------------
    const = ctx.enter_context(tc.tile_pool(name="const", bufs=1))
    work = ctx.enter_context(tc.tile_pool(name="work", bufs=2))
    big = ctx.enter_context(tc.tile_pool(name="big", bufs=1))
    chain = ctx.enter_context(tc.tile_pool(name="chain", bufs=2))
    ps_w = ctx.enter_context(tc.tile_pool(name="ps_w", bufs=1, space="PSUM"))
    ps_bc = ctx.enter_context(tc.tile_pool(name="ps_bc", bufs=2, space="PSUM"))
    ps_a = ctx.enter_context(tc.tile_pool(name="ps_a", bufs=1, space="PSUM"))
    ps_b = ctx.enter_context(tc.tile_pool(name="ps_b", bufs=1, space="PSUM"))

    # ---------------- constants ----------------
    W3f = const.tile([DIN, 3 * DOUT], f32)
    nc.sync.dma_start(out=W3f[:, 0:DOUT], in_=wq_ap)
    nc.sync.dma_start(out=W3f[:, DOUT:2 * DOUT], in_=wk_ap)
    nc.sync.dma_start(out=W3f[:, 2 * DOUT:3 * DOUT], in_=wv_ap)
    W3r = const.tile([DIN, 256], f32r)
    nc.vector.memset(W3r.bitcast(f32), 0.0)
    nc.vector.tensor_copy(out=W3r[:, 0:192], in_=W3f)
    WQKh = const.tile([DIN, 128], f16)        # f16 weights for Q/K stage-2
    nc.vector.tensor_copy(out=WQKh, in_=W3f[:, 0:128])

    IREP = const.tile([128, 64], f16)
    make_identity(nc, IREP[0:64, :])
    make_identity(nc, IREP[64:128, :])

    aI = {}

    def get_aI(val):
        val = float(val)
        if val not in aI:
            t = const.tile([128, WBS], f16, tag=f"aI{len(aI)}",
                           name=f"aI{len(aI)}")
            for p in range(PB):
                nc.vector.tensor_scalar_mul(t[:, ts(p, 64)], IREP, val)
            aI[val] = t
        return aI[val]

    for lv in SCHED_QK + SCHED_V:
        for a, b in lv:
            get_aI(a)
    for c0 in (1.0, 0.25, 0.125):
        get_aI(c0)
    I7f = const.tile([128, WBS], f32)
    for p in range(PB):
        nc.vector.tensor_scalar_mul(I7f[:, ts(p, 64)], IREP, float(2 ** L - 1))
    cI_mm = {}

    def get_cI(val):
        val = float(val)
        if val not in cI_mm:
            t = const.tile([128, 64], f16, tag=f"cImm{len(cI_mm)}",
                           name=f"cImm{len(cI_mm)}")
            nc.vector.tensor_scalar_mul(t, IREP, val)
            cI_mm[val] = t
        return cI_mm[val]

    for v in (1.0, 0.5, 1.0 / 3.0, 2.0, 3.0):
        get_cI(v)
    for lv in SCHED_QK + SCHED_V:
        for a, b in lv:
            get_cI(a / b)
    for v in (0.25, 0.2, 1.0 / 6, 1.0 / 7, 0.125, 1.0 / 9, 0.1,
              1.0 / 11, 1.0 / 12):
        get_cI(v)

    ones_col = const.tile([64, 1], f32)
    nc.vector.memset(ones_col, 1.0)
    ones_col_h = const.tile([64, 1], f16)
    nc.vector.memset(ones_col_h, 16.0)    # folds 1/16 exp prescale into inv
    ones_row = const.tile([1, 64], f32)
    nc.vector.memset(ones_row, 1.0)
    bias_ln = const.tile([64, 1], f32)
    nc.vector.memset(bias_ln, 1.0 + 64e-6)

    # DRAM scratch for partition-moving transposes (DRAM APs unconstrained)
    scrV = nc.dram_tensor("scrV", [64, M * 64], f16, kind="Internal").ap()
    scrM = nc.dram_tensor("scrM", [64, M * 64], f16, kind="Internal").ap()


    def r3(t):
        """[p, (n c)] -> [p, n, 64] view"""
        return t.rearrange("p (n c) -> p n c", c=64)

    def slot(S, f):
        """S [128, (PB,4,64)] -> slot view [128, PB, 64]"""
        return S.rearrange("p (n four c) -> p n four c", four=4, c=64)[:, :, f, :]

    # =====================================================================
    def chain_gen(cn, sched, deg, init_t, b, flat_t):
        """One chain batch: NS chain + corrections + series -> flat_t."""
        ib = r3(init_t[:, ds(b * PB * 64, PB * 64)])     # [128, PB, 64]
        S_cur = None
        Y = Yt = Z = Zt = None       # [128, PB, 64] views
        adj = False
        ACC = None

        for l in range(L):
            steps = sched[l]
            for j, (a, bc) in enumerate(steps):
                Pd = chain.tile([128, WBS], f16, tag=f"Pd{cn}", name=f"Pd{cn}")
                Pd3 = r3(Pd)
                aIt = get_aI(a)
                if j == 0:
                    src = ib if l == 0 else Y
                    rr_stt(Pd3, src, bc, r3(aIt))
                    yield
                else:
                    psW = ps_w.tile([128, WBS], f32, tag="psW", name="psW")
                    psW3 = r3(psW)
                    abI = get_cI(a / bc)
                    Ipat = get_aI(1.0)
                    for h in (0, 1):
                        hs = slice(h * 64, h * 64 + 64)
                        nc.tensor.matmul(psW[hs, :], abI[hs, :], Ipat[hs, :],
                                         start=True, stop=False)
                    for p in range(PB):
                        for h in (0, 1):
                            hs = slice(h * 64, h * 64 + 64)
                            nc.tensor.matmul(psW3[hs, p], Zt[hs, p], Y[hs, p],
                                             start=False,
                                             stop=(p == PB - 1),
                                             skip_group_check=True)
                    yield
                    nc.scalar.activation(out=Pd, in_=psW, func=ACT.Copy,
                                         bias=0.0, scale=float(bc))
                    yield
                # ---- updates ----
                psBC = ps_bc.tile([128, SBW], f32, tag="psBC", name="psBC")
                ps4 = psBC.rearrange("p (n four c) -> p n four c", four=4, c=64)
                YtP = ib if (l == 0 and j == 0) else Yt
                for p in range(PB):
                    for h in (0, 1):
                        hs = slice(h * 64, h * 64 + 64)
                        # Y' = Yt^T P ; Yt' = P^T Yt
                        nc.tensor.matmul(ps4[hs, p, 0, :], YtP[hs, p],
                                         Pd3[hs, p], start=True, stop=True)
                        nc.tensor.matmul(ps4[hs, p, 1, :], Pd3[hs, p],
                                         YtP[hs, p], start=True, stop=True)
                        if j > 0:
                            # Z' = P^T Z ; Zt' = Z^T P
                            nc.tensor.matmul(ps4[hs, p, 2, :], Pd3[hs, p],
                                             Z[hs, p], start=True, stop=True)
                            nc.tensor.matmul(ps4[hs, p, 3, :], Z[hs, p],
                                             Pd3[hs, p], start=True, stop=True)
                yield
                S_new = chain.tile([128, SBW], f16, tag=f"S{cn}", name=f"S{cn}")
                s4 = S_new.rearrange("p (n four c) -> p n four c", four=4, c=64)
                if j == 0:
                    nc.vector.tensor_copy(out=s4[:, :, 0:2, :],
                                          in_=ps4[:, :, 0:2, :])
                    Z = Pd3
                    Zt = Pd3
                    adj = False
                else:
                    nc.vector.tensor_copy(out=S_new[:, 0:SBW // 2],
                                          in_=psBC[:, 0:SBW // 2])
                    nc.scalar.activation(out=S_new[:, SBW // 2:],
                                         in_=psBC[:, SBW // 2:],
                                         func=ACT.Copy, bias=0.0, scale=1.0)
                    Z = slot(S_new, 2)
                    Zt = slot(S_new, 3)
                    adj = True
                Y = slot(S_new, 0)
                Yt = slot(S_new, 1)
                S_cur = S_new
                yield
            # ---- level end correction: psWe = Zt^T Y + Y^T Zt ----
            psWe = ps_w.tile([128, WBS], f32, tag="psW", name="psWe")
            pw3 = r3(psWe)
            for p in range(PB):
                for h in (0, 1):
                    hs = slice(h * 64, h * 64 + 64)
                    nc.tensor.matmul(pw3[hs, p], Zt[hs, p], Y[hs, p],
                                     start=True, stop=False)
                    nc.tensor.matmul(pw3[hs, p], Y[hs, p], Zt[hs, p],
                                     start=False, stop=True)
            yield
            if l == 0:
                ACC = chain.tile([128, WBS], f32, tag=f"acc{cn}",
                                 name=f"acc{cn}")
                rr_stt(ACC, psWe, -0.5, I7f)
            else:
                rr_stt(ACC, psWe, -float(2 ** l) / 2.0, ACC)
            yield
        # ================= series =================
        E = chain.tile([128, WBS], f16, tag=f"E{cn}", name=f"E{cn}")
        rr_stt(r3(E), Y, -1.0, r3(get_aI(1.0)))
        yield
        powers = {1: E}
        for k, rt in ((2, 1), (3, 2), (4, 3)):
            psE = ps_a.tile([128, WBS], f32, tag="scrA", name="psE")
            pe3 = r3(psE)
            e1 = r3(powers[1])
            ert = r3(powers[rt])
            for p in range(PB):
                for h in (0, 1):
                    hs = slice(h * 64, h * 64 + 64)
                    nc.tensor.matmul(pe3[hs, p], e1[hs, p], ert[hs, p],
                                     start=True, stop=True)
            Ek = chain.tile([128, WBS], f16, tag=f"E{k}{cn}", name=f"E{k}{cn}")
            rr_copy(Ek, psE)
            powers[k] = Ek
            yield
        E2, E3, E4 = powers[2], powers[3], powers[4]

        def combo(coefs, dst_tag):
            """PE-accumulated c0 I + c1 E + c2 E2 + c3 E3 + c4 E4 -> f16."""
            psC = ps_a.tile([128, WBS], f32, tag="scrA", name="psC")
            ops = [(get_cI(coefs[0]), get_aI(1.0))] + [
                (get_cI(cv), pw) for cv, pw in
                zip(coefs[1:], (E, E2, E3, E4)) if cv]
            for i, (lh, rh) in enumerate(ops):
                for h in (0, 1):
                    hs = slice(h * 64, h * 64 + 64)
                    nc.tensor.matmul(psC[hs, :], lh[hs, :], rh[hs, :],
                                     start=(i == 0),
                                     stop=(i == len(ops) - 1),
                                     skip_group_check=(i > 0))
            Ct = chain.tile([128, WBS], f16, tag=dst_tag, name=dst_tag)
            rr_copy(Ct, psC)
            return Ct

        C = combo((0.25, 0.2, 1.0 / 6, 1.0 / 7, 0.125), f"C{cn}")
        yield
        if deg == 12:
            C2 = combo((0.125, 1.0 / 9, 0.1, 1.0 / 11, 1.0 / 12), f"C2{cn}")
            psH = ps_a.tile([128, WBS], f32, tag="scrA", name="psH")
            ph3, e43, c23 = r3(psH), r3(E4), r3(C2)
            for p in range(PB):
                for h in (0, 1):
                    hs = slice(h * 64, h * 64 + 64)
                    nc.tensor.matmul(ph3[hs, p], e43[hs, p], c23[hs, p],
                                     start=True, stop=True)
            yield
            CH = chain.tile([128, WBS], f16, tag=f"C2{cn}", name=f"CH{cn}")
            nc.vector.tensor_tensor(out=CH, in0=psH, in1=C, op=OP.add)
            C = CH
            yield
        # psB0 = 1*E + E4@C + (1/2)E2 + (1/3)E3   (accumulated group)
        psB0 = ps_a.tile([128, WBS], f32, tag="scrA", name="psB0")
        e43, c3 = r3(E4), r3(C)
        for h in (0, 1):
            hs = slice(h * 64, h * 64 + 64)
            nc.tensor.matmul(psB0[hs, :], get_cI(1.0)[hs, :], E[hs, :],
                             start=True, stop=False)
        for p in range(PB):
            for h in (0, 1):
                hs = slice(h * 64, h * 64 + 64)
                nc.tensor.matmul(r3(psB0)[hs, p], e43[hs, p], c3[hs, p],
                                 start=False, stop=False,
                                 skip_group_check=True)
        for h in (0, 1):
            hs = slice(h * 64, h * 64 + 64)
            nc.tensor.matmul(psB0[hs, :], get_cI(0.5)[hs, :], E2[hs, :],
                             start=False, stop=False, skip_group_check=True)
            nc.tensor.matmul(psB0[hs, :], get_cI(1.0 / 3.0)[hs, :], E3[hs, :],
                             start=False, stop=True, skip_group_check=True)
        yield
        # LS = -2^L psB0 + ACC -> flat (strided, per h)
        fl3 = flat_t.rearrange("p (pr two c) -> p pr two c", two=2, c=64)
        acc3 = r3(ACC)
        for h in (0, 1):
            hs = slice(h * 64, h * 64 + 64)
            rr_stt(fl3[:, ds(b * PB, PB), h, :], r3(psB0)[hs], -float(2 ** L),
                   acc3[hs])
        yield

    # ======================= per-sample pipeline =========================
    for s in range(nsamp):
        initQ = work.tile([128, NP_PAIR * 64], f16, tag="initQ", name="initQ")
        initK = work.tile([128, NP_PAIR * 64], f16, tag="initK", name="initK")
        initV = work.tile([128, NP_PAIR * 64], f16, tag="initV", name="initV")

        # ---------------- congruence ----------------
        for g in range(8):          # 8 matrices per group
            if g % 2 == 0:
                xbuf = work.tile([DIN, 16 * DIN], f32r, tag="xbuf", name="xbuf")
                nc.gpsimd.dma_start(
                    out=xbuf.rearrange("p (i c) -> p i c", c=DIN),
                    in_=x_ap[s, ds(g * 8, 16)].rearrange("i p c -> p i c"))
            pqks = []
            pvs = []
            for r in range(2):      # 2 rounds x 4 matrices
                ps1 = ps_b.tile([DIN, 4 * 256], f32, tag="scrB", name="ps1")
                for mi in range(4):
                    mg = (g % 2) * 8 + r * 4 + mi
                    nc.tensor.matmul(ps1[:, ts(mi, 256)],
                                     xbuf[:, ts(mg, DIN)], W3r,
                                     start=True, stop=True)
                pqk = work.tile([DIN, 4 * 128], f16, tag="pqk", name="pqk")
                pv = work.tile([DIN, 4 * 64], f32, tag="pv", name="pv")
                rr_copy(pqk.rearrange("p (n c) -> p n c", c=128),
                        ps1.rearrange("p (n c) -> p n c", c=256)[:, :, 0:128])
                rr_copy(pv.rearrange("p (n c) -> p n c", c=64),
                        ps1.rearrange("p (n c) -> p n c", c=256)[:, :, 128:192])
                pqks.append(pqk)
                pvs.append(pv)
            for wi, init_t in ((0, initQ), (1, initK), (2, initV)):
                psI = ps_w.tile([128, WBS], f32, tag="psW", name="psI")
                for m in range(8):
                    r, mi = m // 4, m % 4
                    pr, h = m // 2, m % 2
                    hs = slice(h * 64, h * 64 + 64)
                    if wi < 2:
                        rhs = pqks[r][:, mi * 128 + wi * 64:
                                      mi * 128 + wi * 64 + 64]
                    else:
                        rhs = pvs[r][:, ts(mi, 64)]
                    lhsW = (WQKh[:, ts(wi, 64)] if wi < 2
                            else W3f[:, ts(2, 64)])
                    nc.tensor.matmul(psI[hs, ts(pr, 64)], lhsW, rhs,
                                     start=True, stop=True)
                rr_copy(init_t[:, ds(g * 4 * 64, WBS)], psI, scale=1.0 / C_NORM)

        if s == 0:
            tap("initQ", initQ)
            tap("initK", initK)
            tap("initV", initV)
        # ---------------- chains ----------------
        flatQ = big.tile([64, M * 64], f16, tag="flatQ", name="flatQ")
        flatK = big.tile([64, M * 64], f16, tag="flatK", name="flatK")
        flatV = big.tile([64, M * 64], f16, tag="flatV", name="flatV")
        for b in range(NBATCH):
            gens = [chain_gen("q", SCHED_QK, DEG_QK, initQ, b, flatQ),
                    chain_gen("k", SCHED_QK, DEG_QK, initK, b, flatK),
                    chain_gen("v", SCHED_V, DEG_V, initV, b, flatV)]
            while gens:
                gens = [g for g in gens
                        if next(g, StopIteration) is not StopIteration]

        if s == 0:
            tap("flatQ", flatQ)
            tap("flatK", flatK)
            tap("flatV", flatV)
        # ---------------- attention ----------------
        partQ = work.tile([64, M], f32, tag="partQ", name="partQ")
        partK = work.tile([64, M], f32, tag="partK", name="partK")
        for flat_t, part_t in ((flatQ, partQ), (flatK, partK)):
            sq = big.tile([64, M * 64], f16, tag="sqscr", name="sqscr")
            nc.vector.tensor_mul(sq, flat_t, flat_t)
            nc.vector.tensor_reduce(
                out=part_t, in_=sq.rearrange("p (i c) -> p i c", c=64),
                axis=AX.X, op=OP.add)
        ps_qn = ps_a.tile([1, 64], f32, tag="scrA", name="ps_qn")
        nc.tensor.matmul(ps_qn, ones_col, partQ, start=True, stop=True)
        qn_row = work.tile([1, 64], f32, tag="qnrow", name="qnrow")
        nc.vector.tensor_copy(out=qn_row, in_=ps_qn)
        ps_kn = ps_a.tile([64, 1], f32, tag="scrA", name="ps_kn")
        nc.tensor.matmul(ps_kn, partK, ones_col, start=True, stop=True)
        kn_col = work.tile([64, 1], f32, tag="kncol", name="kncol")
        nc.vector.tensor_copy(out=kn_col, in_=ps_kn)
        ps_qrep = ps_a.tile([64, 64], f32, tag="scrA", name="ps_qrep")
        nc.tensor.matmul(ps_qrep, ones_row, qn_row, start=True, stop=True)
        qrep = work.tile([64, 64], f32, tag="qrep", name="qrep")
        nc.vector.tensor_copy(out=qrep, in_=ps_qrep)

        ps_cross = ps_a.tile([64, 64], f32, tag="scrA", name="ps_cross")
        fQ3 = flatQ.rearrange("p (i c) -> p c i", c=64)
        fK3 = flatK.rearrange("p (i c) -> p c i", c=64)
        for c in range(64):
            nc.tensor.matmul(ps_cross, fK3[:, c, :], fQ3[:, c, :],
                             start=(c == 0), stop=(c == 63))
        Et = work.tile([64, 64], f32, tag="Et", name="Et")
        nc.vector.scalar_tensor_tensor(out=Et, in0=ps_cross, scalar=-2.0,
                                       in1=qrep, op0=OP.mult, op1=OP.add)
        nc.vector.tensor_scalar(out=Et, in0=Et, scalar1=kn_col, scalar2=0.0,
                                op0=OP.add, op1=OP.max)
        lnE = work.tile([64, 64], f32, tag="lnE", name="lnE")
        nc.scalar.activation(out=lnE, in_=Et, func=ACT.Ln,
                             bias=bias_ln, scale=1.0)
        ln1 = work.tile([64, 64], f32, tag="ln1", name="ln1")
        nc.vector.tensor_scalar_add(ln1, lnE, 1.0)
        sc = work.tile([64, 64], f32, tag="sc", name="sc")
        nc.vector.reciprocal(out=sc, in_=ln1)
        expS = work.tile([64, 64], f16, tag="expS", name="expS")
        nc.scalar.activation(out=expS, in_=sc, func=ACT.Exp, bias=0.0,
                             scale=1.0)
        ps_cs = ps_a.tile([64, 1], f32, tag="scrA", name="ps_cs")
        nc.tensor.matmul(ps_cs, expS, ones_col_h, start=True, stop=True)
        inv = work.tile([64, 1], f32, tag="inv", name="inv")
        nc.vector.reciprocal(out=inv, in_=ps_cs)

        # VF: flatV [p, (i c)] -> VF [i, (p c)] via DRAM roundtrip
        VF = big.tile([64, M * 64], f16, tag="VF", name="VF")
        nc.sync.dma_start(out=scrV, in_=flatV)
        nc.sync.dma_start(
            out=VF.rearrange("i (p c) -> i p c", c=64),
            in_=scrV.rearrange("p (i c) -> i p c", c=64))
        if s == 0:
            tap("VF", VF)
        # mixing: M2[j, (p c)] = sum_i expS[i, j] VF[i, (p c)] * inv[j]
        M2 = big.tile([64, M * 64], f16, tag="M2", name="M2")
        for ch in range(4):
            ps_m = ps_b.tile([64, 1024], f32, tag="scrB", name="ps_m")
            nc.tensor.matmul(ps_m[:, 0:512], expS, VF[:, ds(ch * 1024, 512)],
                             start=True, stop=True)
            nc.tensor.matmul(ps_m[:, 512:1024], expS,
                             VF[:, ds(ch * 1024 + 512, 512)],
                             start=True, stop=True)
            nc.vector.tensor_scalar_mul(M2[:, ds(ch * 1024, 1024)], ps_m, inv)
        # S1M scatter: M2 [j=(pr h), (p c)] -> S1M [(h p), (pr c)] via DRAM
        S1M = big.tile([128, NP_PAIR * 64], f16, tag="S1M", name="S1M")
        nc.sync.dma_start(out=scrM, in_=M2)
        for h in (0, 1):
            nc.sync.dma_start(
                out=S1M[h * 64:(h + 1) * 64, :].rearrange(
                    "p (pr c) -> p pr c", c=64),
                in_=scrM.rearrange("(pr two) (p c) -> two p pr c",
                                   two=2, c=64)[h])

        if s == 0:
            tap("M2", M2)
            tap("S1M", S1M)
            tap("expS", expS)
        # ---------------- exp ----------------
        outS1 = big.tile([128, NP_PAIR * 64], f32, tag="outS1", name="outS1")
        for b in range(NBATCH):
            Xs = S1M[:, ds(b * WBS, WBS)]
            X3 = r3(Xs)
            H = chain.tile([128, WBS], f16, tag="expH", name="expH")
            rr_stt(H, Xs, 1.0 / EXP_DEG, get_aI(1.0))
            for k in range(EXP_DEG - 1, 0, -1):
                psx = ps_a.tile([128, WBS], f32, tag="scrA", name="psx")
                px3, h3 = r3(psx), r3(H)
                for hh in (0, 1):
                    hs = slice(hh * 64, hh * 64 + 64)
                    nc.tensor.matmul(psx[hs, :], get_cI(float(k))[hs, :],
                                     get_aI(1.0)[hs, :],
                                     start=True, stop=False)
                for p in range(PB):
                    for hh in (0, 1):
                        hs = slice(hh * 64, hh * 64 + 64)
                        nc.tensor.matmul(px3[hs, p], X3[hs, p], h3[hs, p],
                                         start=False, stop=(p == PB - 1),
                                         skip_group_check=True)
                H2 = chain.tile([128, WBS], f16, tag="expH", name="expH2")
                rr_copy(H2, psx, scale=1.0 / k)
                H = H2
            for sq_i in range(EXP_SQ):
                psx = ps_a.tile([128, WBS], f32, tag="scrA", name="psx2")
                px3, h3 = r3(psx), r3(H)
                for p in range(PB):
                    for hh in (0, 1):
                        hs = slice(hh * 64, hh * 64 + 64)
                        nc.tensor.matmul(px3[hs, p], h3[hs, p], h3[hs, p],
                                         start=True, stop=True)
                if sq_i < EXP_SQ - 1:
                    H2 = chain.tile([128, WBS], f16, tag="expH", name="expH3")
                    rr_copy(H2, psx)
                    H = H2
                else:
                    rr_copy(outS1[:, ds(b * WBS, WBS)], psx, scale=C_NORM)

        o3 = out_ap[s].rearrange("(pr two) r c -> two r pr c", two=2)
        nc.sync.dma_start(
            out=o3[0],
            in_=outS1[0:64, :].rearrange("p (pr c) -> p pr c", c=64))
        nc.sync.dma_start(
            out=o3[1],
            in_=outS1[64:128, :].rearrange("p (pr c) -> p pr c", c=64))


def build(nsamp=NSAMP, num_devices=NCORES, debug_taps=False):
    import concourse.bacc as bacc
    import concourse.mybir as mybir
    import concourse.tile as tile

    nc = bacc.Bacc("TRN2", target_bir_lowering=False, debug=False,
                   num_devices=num_devices)
    f32 = mybir.dt.float32
    x_ap = nc.dram_tensor("x", [nsamp, M, DIN, DIN], f32,
                          kind="ExternalInput").ap()
    wq = nc.dram_tensor("wq", [DIN, DOUT], f32, kind="ExternalInput").ap()
    wk = nc.dram_tensor("wk", [DIN, DOUT], f32, kind="ExternalInput").ap()
    wv = nc.dram_tensor("wv", [DIN, DOUT], f32, kind="ExternalInput").ap()
    out = nc.dram_tensor("out", [nsamp, M, DOUT, DOUT], f32,
                         kind="ExternalOutput").ap()

    taps = {}
    if debug_taps:
        for nm, shp, dt_ in (("initQ", [128, 2048], mybir.dt.float16),
                             ("initK", [128, 2048], mybir.dt.float16),
                             ("initV", [128, 2048], mybir.dt.float16),
                             ("flatQ", [64, 4096], mybir.dt.float16),
                             ("flatK", [64, 4096], mybir.dt.float16),
                             ("flatV", [64, 4096], mybir.dt.float16),
                             ("M2", [64, 4096], mybir.dt.float16),
                             ("VF", [64, 4096], mybir.dt.float16),
                             ("S1M", [128, 2048], mybir.dt.float16),
                             ("expS", [64, 64], mybir.dt.float16)):
            taps[nm] = nc.dram_tensor("tap_" + nm, shp, dt_,
                                      kind="ExternalOutput").ap()
    with tile.TileContext(nc) as tc, ExitStack() as ctx:
        emit_kernel(nc, tc, ctx, x_ap, wq, wk, wv, out, nsamp=nsamp,
                    taps=taps if debug_taps else None)
    nc.compile()
    return nc


_CACHED = {}


def _get_nc(nsamp):
    from concourse.bass_interp import get_hw_module
    if nsamp not in _CACHED:
        nc = build(nsamp=nsamp)
        nc.m = get_hw_module(nc.m)
        _CACHED[nsamp] = nc
    return _CACHED[nsamp]


def kernel(x, Wq, Wk, Wv):
    from concourse.bass_utils import run_bass_kernel_spmd

    bs = x.shape[0]
    nsamp = bs // NCORES
    nc = _get_nc(nsamp)
    in_maps = []
    for c in range(NCORES):
        in_maps.append({
            "x": np.ascontiguousarray(x[c * nsamp:(c + 1) * nsamp],
                                      dtype=np.float32),
            "wq": np.ascontiguousarray(Wq, dtype=np.float32),
            "wk": np.ascontiguousarray(Wk, dtype=np.float32),
            "wv": np.ascontiguousarray(Wv, dtype=np.float32),
        })
    res = run_bass_kernel_spmd(nc, in_maps, list(range(NCORES)))
    outs = [res.results[c]["out"] for c in range(NCORES)]
    full = np.concatenate(outs, axis=0)
    return full.reshape(bs * M, DOUT, DOUT).astype(np.float32)


# revision 16
# speedup vs baseline: 1.8922x; 1.1467x over previous
"""Trainium2 Bass kernel for nn_AttentionManifold (SPD manifold attention).

For each of bs*m=2048 SPD matrices X (100x100): Q/K/V = W^T X W (64x64),
logQ/K/V = matrix log, log-Euclidean attention, mixed = prob-weighted sum
of logV, out = matrix exp(mixed).

Matrix log: Newton-Schulz coupled sqrt chain, L=3 levels, with
first-order level corrections  log A = 2^L log Y_L - sum_l 2^l log W_l,
log W ~= -(I - W);  series log(Y) via deg-8/12 Paterson-Stockmeyer.

Chain scheme '2s_approx' (emulator-validated): exact-transpose pairs
(Y, Yt, Z, Zt) in fp16, P = aI + bW only (no Pt):
    W    = {lhsT=Zt_h, rhs=Y_h}              (per matrix)
    P    = aI + b psW                        (stt)
    Y'   = {lhsT=Yt_h, rhs=P_h}              = Y P
    [Yt'|Z'] = {lhsT=bd(P), rhs=[Yt|Z]}      = [P^T Yt | P^T Z]  (packed)
    Zt'  = {lhsT=Z_h, rhs=P_h}               = (P^T Z)^T bitwise
Every W is a congruence of the SPD input => fp16-stable.

exp via scaling-squaring (deg-4 Horner, 4 squarings).
Sharding: pure data parallelism, bs=32 -> 4 samples per NeuronCore.
"""
import numpy as np
from contextlib import ExitStack

C_NORM = 16.0
BS, M, DIN, DOUT = 32, 64, 100, 64
NCORES = 8
NSAMP = BS // NCORES
NP_PAIR = M // 2          # 32 pairs per sample
PB = 4                    # pairs per chain batch
NBATCH = NP_PAIR // PB    # 8

SCHED_V = [
    [(8.965874126, -13.460097634), (2.380408822, -0.250737931),
     (2.380408822, -0.250532192), (2.380408822, -0.250326648),
     (0.861964497, -0.071654452), (1.542284382, -0.519941516)],
    [(7.758850039, -8.666077201), (0.987610378, -0.093162713),
     (1.645967366, -0.5826622), (1.507505828, -0.502426376)],
    [(6.551825952, -5.22018671), (0.65339645, -0.038866921)],
]
SCHED_QK = [
    [(8.965874126, -13.460097634), (2.380408822, -0.250737931),
     (2.380408822, -0.250532192), (2.380408822, -0.250326648),
     (0.861964497, -0.071654452), (1.542284382, -0.519941516)],
    [(7.758850039, -8.666077201), (0.987610378, -0.093162713),
     (1.645967366, -0.5826622)],
    [(6.404040404, -4.899837718), (0.670769231, -0.04132838)],
]
DEG_QK = 8
DEG_V = 8
EXP_DEG = 4
EXP_SQ = 4
L = 3
WBS = PB * 64             # 256
SBW = PB * 256            # 1024


def emit_kernel(nc, tc, ctx, x_ap, wq_ap, wk_ap, wv_ap, out_ap, nsamp=NSAMP,
                taps=None):
    def tap(name, t):
        if taps is not None and name in taps:
            nc.sync.dma_start(out=taps[name], in_=t)

    import concourse.mybir as mybir
    from concourse.bass import ds, ts
    from concourse.masks import make_identity

    f32 = mybir.dt.float32
    f32r = mybir.dt.float32r
    f16 = mybir.dt.float16
    AX = mybir.AxisListType
    OP = mybir.AluOpType
    ACT = mybir.ActivationFunctionType

    # ---- engine rotation helpers ----
    _rrc = [0]
    _rrs = [0]

    from concourse.bass import MemorySpace as _MS

    def _psum(*aps):
        return any(a.space == _MS.PSUM for a in aps)

    def rr_copy(out, in_, scale=None):
        pool = ((nc.vector, nc.scalar) if _psum(out, in_)
                else (nc.vector, nc.scalar, nc.gpsimd))
        e = pool[_rrc[0] % len(pool)]
        _rrc[0] += 1
        if e is nc.scalar:
            nc.scalar.activation(out=out, in_=in_, func=ACT.Copy, bias=0.0,
                                 scale=1.0 if scale is None else float(scale))
        elif scale is None:
            e.tensor_copy(out=out, in_=in_)
        else:
            e.tensor_scalar_mul(out, in_, float(scale))

    def rr_stt(out, in0, scalar, in1):
        nc.vector.scalar_tensor_tensor(out=out, in0=in0, scalar=float(scalar),
                                       in1=in1, op0=OP.mult, op1=OP.add)

    # ---------------- pools ----------------
    const = ctx.enter_context(tc.tile_pool(name="const", bufs=1))
    work = ctx.enter_context(tc.tile_pool(name="work", bufs=2))
    big = ctx.enter_context(tc.tile_pool(name="big", bufs=1))
    chain = ctx.enter_context(tc.tile_pool(name="chain", bufs=2))
    ps_w = ctx.enter_context(tc.tile_pool(name="ps_w", bufs=1, space="PSUM"))
    ps_bc = ctx.enter_context(tc.tile_pool(name="ps_bc", bufs=2, space="PSUM"))
    ps_a = ctx.enter_context(tc.tile_pool(name="ps_a", bufs=1, space="PSUM"))
    ps_b = ctx.enter_context(tc.tile_pool(name="ps_b", bufs=1, space="PSUM"))

    # ---------------- constants ----------------
    W3f = const.tile([DIN, 3 * DOUT], f32)
    nc.sync.dma_start(out=W3f[:, 0:DOUT], in_=wq_ap)
    nc.sync.dma_start(out=W3f[:, DOUT:2 * DOUT], in_=wk_ap)
    nc.sync.dma_start(out=W3f[:, 2 * DOUT:3 * DOUT], in_=wv_ap)
    W3r = const.tile([DIN, 256], f32r)
    nc.vector.memset(W3r.bitcast(f32), 0.0)
    nc.vector.tensor_copy(out=W3r[:, 0:192], in_=W3f)
    WQKh = const.tile([DIN, 128], f16)        # f16 weights for Q/K stage-2
    nc.vector.tensor_copy(out=WQKh, in_=W3f[:, 0:128])

    IREP = const.tile([128, 64], f16)
    make_identity(nc, IREP[0:64, :])
    make_identity(nc, IREP[64:128, :])

    aI = {}

    def get_aI(val):
        val = float(val)
        if val not in aI:
            t = const.tile([128, WBS], f16, tag=f"aI{len(aI)}",
                           name=f"aI{len(aI)}")
            for p in range(PB):
                nc.vector.tensor_scalar_mul(t[:, ts(p, 64)], IREP, val)
            aI[val] = t
        return aI[val]

    for lv in SCHED_QK + SCHED_V:
        for a, b in lv:
            get_aI(a)
    for c0 in (1.0, 0.25, 0.125):
        get_aI(c0)
    I7f = const.tile([128, WBS], f32)
    for p in range(PB):
        nc.vector.tensor_scalar_mul(I7f[:, ts(p, 64)], IREP, float(2 ** L - 1))
    cI_mm = {}

    def get_cI(val):
        val = float(val)
        if val not in cI_mm:
            t = const.tile([128, 64], f16, tag=f"cImm{len(cI_mm)}",
                           name=f"cImm{len(cI_mm)}")
            nc.vector.tensor_scalar_mul(t, IREP, val)
            cI_mm[val] = t
        return cI_mm[val]

    for v in (1.0, 0.5, 1.0 / 3.0, 2.0, 3.0):
        get_cI(v)
    for lv in SCHED_QK + SCHED_V:
        for a, b in lv:
            get_cI(a / b)
    for v in (0.25, 0.2, 1.0 / 6, 1.0 / 7, 0.125, 1.0 / 9, 0.1,
              1.0 / 11, 1.0 / 12):
        get_cI(v)

    ones_col = const.tile([64, 1], f32)
    nc.vector.memset(ones_col, 1.0)
    ones_col_h = const.tile([64, 1], f16)
    nc.vector.memset(ones_col_h, 16.0)    # folds 1/16 exp prescale into inv
    ones_row = const.tile([1, 64], f32)
    nc.vector.memset(ones_row, 1.0)
    bias_ln = const.tile([64, 1], f32)
    nc.vector.memset(bias_ln, 1.0 + 64e-6)

    # DRAM scratch for partition-moving transposes (DRAM APs unconstrained)
    scrV = nc.dram_tensor("scrV", [64, M * 64], f16, kind="Internal").ap()
    scrM = nc.dram_tensor("scrM", [64, M * 64], f16, kind="Internal").ap()


    def r3(t):
        """[p, (n c)] -> [p, n, 64] view"""
        return t.rearrange("p (n c) -> p n c", c=64)

    def slot(S, f):
        """S [128, (PB,4,64)] -> slot view [128, PB, 64]"""
        return S.rearrange("p (n four c) -> p n four c", four=4, c=64)[:, :, f, :]

    # =====================================================================
    def chain_gen(cn, sched, deg, init_t, b, flat_t):
        """One chain batch: NS chain + corrections + series -> flat_t."""
        ib = r3(init_t[:, ds(b * PB * 64, PB * 64)])     # [128, PB, 64]
        S_cur = None
        Y = Yt = Z = Zt = None       # [128, PB, 64] views
        adj = False
        ACC = None

        for l in range(L):
            steps = sched[l]
            for j, (a, bc) in enumerate(steps):
                Pd = chain.tile([128, WBS], f16, tag=f"Pd{cn}", name=f"Pd{cn}")
                Pd3 = r3(Pd)
                aIt = get_aI(a)
                if j == 0:
                    src = ib if l == 0 else Y
                    rr_stt(Pd3, src, bc, r3(aIt))
                    yield
                else:
                    psW = ps_w.tile([128, WBS], f32, tag="psW", name="psW")
                    psW3 = r3(psW)
                    abI = get_cI(a / bc)
                    Ipat = get_aI(1.0)
                    for h in (0, 1):
                        hs = slice(h * 64, h * 64 + 64)
                        nc.tensor.matmul(psW[hs, :], abI[hs, :], Ipat[hs, :],
                                         start=True, stop=False)
                    for p in range(PB):
                        for h in (0, 1):
                            hs = slice(h * 64, h * 64 + 64)
                            nc.tensor.matmul(psW3[hs, p], Zt[hs, p], Y[hs, p],
                                             start=False,
                                             stop=(p == PB - 1),
                                             skip_group_check=True)
                    yield
                    nc.scalar.activation(out=Pd, in_=psW, func=ACT.Copy,
                                         bias=0.0, scale=float(bc))
                    yield
                # ---- updates ----
                psBC = ps_bc.tile([128, SBW], f32, tag="psBC", name="psBC")
                ps4 = psBC.rearrange("p (n four c) -> p n four c", four=4, c=64)
                YtP = ib if (l == 0 and j == 0) else Yt
                for p in range(PB):
                    for h in (0, 1):
                        hs = slice(h * 64, h * 64 + 64)
                        # Y' = Yt^T P ; Yt' = P^T Yt
                        nc.tensor.matmul(ps4[hs, p, 0, :], YtP[hs, p],
                                         Pd3[hs, p], start=True, stop=True)
                        nc.tensor.matmul(ps4[hs, p, 1, :], Pd3[hs, p],
                                         YtP[hs, p], start=True, stop=True)
                        if j > 0:
                            # Z' = P^T Z ; Zt' = Z^T P
                            nc.tensor.matmul(ps4[hs, p, 2, :], Pd3[hs, p],
                                             Z[hs, p], start=True, stop=True)
                            nc.tensor.matmul(ps4[hs, p, 3, :], Z[hs, p],
                                             Pd3[hs, p], start=True, stop=True)
                yield
                S_new = chain.tile([128, SBW], f16, tag=f"S{cn}", name=f"S{cn}")
                s4 = S_new.rearrange("p (n four c) -> p n four c", four=4, c=64)
                if j == 0:
                    nc.vector.tensor_copy(out=s4[:, :, 0:2, :],
                                          in_=ps4[:, :, 0:2, :])
                    Z = Pd3
                    Zt = Pd3
                    adj = False
                else:
                    nc.vector.tensor_copy(out=S_new[:, 0:SBW // 2],
                                          in_=psBC[:, 0:SBW // 2])
                    nc.scalar.activation(out=S_new[:, SBW // 2:],
                                         in_=psBC[:, SBW // 2:],
                                         func=ACT.Copy, bias=0.0, scale=1.0)
                    Z = slot(S_new, 2)
                    Zt = slot(S_new, 3)
                    adj = True
                Y = slot(S_new, 0)
                Yt = slot(S_new, 1)
                S_cur = S_new
                yield
            # ---- level end correction: psWe = Zt^T Y + Y^T Zt ----
            psWe = ps_w.tile([128, WBS], f32, tag="psW", name="psWe")
            pw3 = r3(psWe)
            for p in range(PB):
                for h in (0, 1):
                    hs = slice(h * 64, h * 64 + 64)
                    nc.tensor.matmul(pw3[hs, p], Zt[hs, p], Y[hs, p],
                                     start=True, stop=False)
                    nc.tensor.matmul(pw3[hs, p], Y[hs, p], Zt[hs, p],
                                     start=False, stop=True)
            yield
            if l == 0:
                ACC = chain.tile([128, WBS], f32, tag=f"acc{cn}",
                                 name=f"acc{cn}")
                rr_stt(ACC, psWe, -0.5, I7f)
            else:
                rr_stt(ACC, psWe, -float(2 ** l) / 2.0, ACC)
            yield
        # ================= series =================
        E = chain.tile([128, WBS], f16, tag=f"E{cn}", name=f"E{cn}")
        rr_stt(r3(E), Y, -1.0, r3(get_aI(1.0)))
        yield
        powers = {1: E}
        for k, rt in ((2, 1), (3, 2), (4, 3)):
            psE = ps_a.tile([128, WBS], f32, tag="scrA", name="psE")
            pe3 = r3(psE)
            e1 = r3(powers[1])
            ert = r3(powers[rt])
            for p in range(PB):
                for h in (0, 1):
                    hs = slice(h * 64, h * 64 + 64)
                    nc.tensor.matmul(pe3[hs, p], e1[hs, p], ert[hs, p],
                                     start=True, stop=True)
            Ek = chain.tile([128, WBS], f16, tag=f"E{k}{cn}", name=f"E{k}{cn}")
            rr_copy(Ek, psE)
            powers[k] = Ek
            yield
        E2, E3, E4 = powers[2], powers[3], powers[4]

        def combo(coefs, dst_tag):
            """PE-accumulated c0 I + c1 E + c2 E2 + c3 E3 + c4 E4 -> f16."""
            psC = ps_a.tile([128, WBS], f32, tag="scrA", name="psC")
            ops = [(get_cI(coefs[0]), get_aI(1.0))] + [
                (get_cI(cv), pw) for cv, pw in
                zip(coefs[1:], (E, E2, E3, E4)) if cv]
            for i, (lh, rh) in enumerate(ops):
                for h in (0, 1):
                    hs = slice(h * 64, h * 64 + 64)
                    nc.tensor.matmul(psC[hs, :], lh[hs, :], rh[hs, :],
                                     start=(i == 0),
                                     stop=(i == len(ops) - 1),
                                     skip_group_check=(i > 0))
            Ct = chain.tile([128, WBS], f16, tag=dst_tag, name=dst_tag)
            rr_copy(Ct, psC)
            return Ct

        C = combo((0.25, 0.2, 1.0 / 6, 1.0 / 7, 0.125), f"C{cn}")
        yield
        if deg == 12:
            C2 = combo((0.125, 1.0 / 9, 0.1, 1.0 / 11, 1.0 / 12), f"C2{cn}")
            psH = ps_a.tile([128, WBS], f32, tag="scrA", name="psH")
            ph3, e43, c23 = r3(psH), r3(E4), r3(C2)
            for p in range(PB):
                for h in (0, 1):
                    hs = slice(h * 64, h * 64 + 64)
                    nc.tensor.matmul(ph3[hs, p], e43[hs, p], c23[hs, p],
                                     start=True, stop=True)
            yield
            CH = chain.tile([128, WBS], f16, tag=f"C2{cn}", name=f"CH{cn}")
            nc.vector.tensor_tensor(out=CH, in0=psH, in1=C, op=OP.add)
            C = CH
            yield
        # psB0 = 1*E + E4@C + (1/2)E2 + (1/3)E3   (accumulated group)
        psB0 = ps_a.tile([128, WBS], f32, tag="scrA", name="psB0")
        e43, c3 = r3(E4), r3(C)
        for h in (0, 1):
            hs = slice(h * 64, h * 64 + 64)
            nc.tensor.matmul(psB0[hs, :], get_cI(1.0)[hs, :], E[hs, :],
                             start=True, stop=False)
        for p in range(PB):
            for h in (0, 1):
                hs = slice(h * 64, h * 64 + 64)
                nc.tensor.matmul(r3(psB0)[hs, p], e43[hs, p], c3[hs, p],
                                 start=False, stop=False,
                                 skip_group_check=True)
        for h in (0, 1):
            hs = slice(h * 64, h * 64 + 64)
            nc.tensor.matmul(psB0[hs, :], get_cI(0.5)[hs, :], E2[hs, :],
                             start=False, stop=False, skip_group_check=True)
            nc.tensor.matmul(psB0[hs, :], get_cI(1.0 / 3.0)[hs, :], E3[hs, :],
                             start=False, stop=True, skip_group_check=True)
        yield
        # LS = -2^L psB0 + ACC -> flat (strided, per h)
        fl3 = flat_t.rearrange("p (pr two c) -> p pr two c", two=2, c=64)
        acc3 = r3(ACC)
        for h in (0, 1):
            hs = slice(h * 64, h * 64 + 64)
            rr_stt(fl3[:, ds(b * PB, PB), h, :], r3(psB0)[hs], -float(2 ** L),
                   acc3[hs])
        yield

    # ======================= per-sample pipeline =========================
    for s in range(nsamp):
        initQ = work.tile([128, NP_PAIR * 64], f16, tag="initQ", name="initQ")
        initK = work.tile([128, NP_PAIR * 64], f16, tag="initK", name="initK")
        initV = work.tile([128, NP_PAIR * 64], f16, tag="initV", name="initV")

        # ---------------- congruence ----------------
        for g in range(8):          # 8 matrices per group
            if g % 2 == 0:
                xbuf = work.tile([DIN, 16 * DIN], f32r, tag="xbuf", name="xbuf")
                nc.gpsimd.dma_start(
                    out=xbuf.rearrange("p (i c) -> p i c", c=DIN),
                    in_=x_ap[s, ds(g * 8, 16)].rearrange("i p c -> p i c"))
            pqks = []
            pvs = []
            for r in range(2):      # 2 rounds x 4 matrices
                ps1 = ps_b.tile([DIN, 4 * 256], f32, tag="scrB", name="ps1")
                for mi in range(4):
                    mg = (g % 2) * 8 + r * 4 + mi
                    nc.tensor.matmul(ps1[:, ts(mi, 256)],
                                     xbuf[:, ts(mg, DIN)], W3r,
                                     start=True, stop=True)
                pqk = work.tile([DIN, 4 * 128], f16, tag="pqk", name="pqk")
                pv = work.tile([DIN, 4 * 64], f32, tag="pv", name="pv")
                rr_copy(pqk.rearrange("p (n c) -> p n c", c=128),
                        ps1.rearrange("p (n c) -> p n c", c=256)[:, :, 0:128])
                rr_copy(pv.rearrange("p (n c) -> p n c", c=64),
                        ps1.rearrange("p (n c) -> p n c", c=256)[:, :, 128:192])
                pqks.append(pqk)
                pvs.append(pv)
            for wi, init_t in ((0, initQ), (1, initK), (2, initV)):
                psI = ps_w.tile([128, WBS], f32, tag="psW", name="psI")
                for m in range(8):
                    r, mi = m // 4, m % 4
                    pr, h = m // 2, m % 2
                    hs = slice(h * 64, h * 64 + 64)
                    if wi < 2:
                        rhs = pqks[r][:, mi * 128 + wi * 64:
                                      mi * 128 + wi * 64 + 64]
                    else:
                        rhs = pvs[r][:, ts(mi, 64)]
                    lhsW = (WQKh[:, ts(wi, 64)] if wi < 2
                            else W3f[:, ts(2, 64)])
                    nc.tensor.matmul(psI[hs, ts(pr, 64)], lhsW, rhs,
                                     start=True, stop=True)
                rr_copy(init_t[:, ds(g * 4 * 64, WBS)], psI, scale=1.0 / C_NORM)

        if s == 0:
            tap("initQ", initQ)
            tap("initK", initK)
            tap("initV", initV)
        # ---------------- chains ----------------
        flatQ = big.tile([64, M * 64], f16, tag="flatQ", name="flatQ")
        flatK = big.tile([64, M * 64], f16, tag="flatK", name="flatK")
        flatV = big.tile([64, M * 64], f16, tag="flatV", name="flatV")
        for b in range(NBATCH):
            gens = [chain_gen("q", SCHED_QK, DEG_QK, initQ, b, flatQ),
                    chain_gen("k", SCHED_QK, DEG_QK, initK, b, flatK),
                    chain_gen("v", SCHED_V, DEG_V, initV, b, flatV)]
            while gens:
                gens = [g for g in gens
                        if next(g, StopIteration) is not StopIteration]

        if s == 0:
            tap("flatQ", flatQ)
            tap("flatK", flatK)
            tap("flatV", flatV)
        # ---------------- attention ----------------
        partQ = work.tile([64, M], f32, tag="partQ", name="partQ")
        partK = work.tile([64, M], f32, tag="partK", name="partK")
        for flat_t, part_t in ((flatQ, partQ), (flatK, partK)):
            sq = big.tile([64, M * 64], f16, tag="sqscr", name="sqscr")
            nc.vector.tensor_mul(sq, flat_t, flat_t)
            nc.vector.tensor_reduce(
                out=part_t, in_=sq.rearrange("p (i c) -> p i c", c=64),
                axis=AX.X, op=OP.add)
        ps_qn = ps_a.tile([1, 64], f32, tag="scrA", name="ps_qn")
        nc.tensor.matmul(ps_qn, ones_col, partQ, start=True, stop=True)
        qn_row = work.tile([1, 64], f32, tag="qnrow", name="qnrow")
        nc.vector.tensor_copy(out=qn_row, in_=ps_qn)
        ps_kn = ps_a.tile([64, 1], f32, tag="scrA", name="ps_kn")
        nc.tensor.matmul(ps_kn, partK, ones_col, start=True, stop=True)
        kn_col = work.tile([64, 1], f32, tag="kncol", name="kncol")
        nc.vector.tensor_copy(out=kn_col, in_=ps_kn)
        ps_qrep = ps_a.tile([64, 64], f32, tag="scrA", name="ps_qrep")
        nc.tensor.matmul(ps_qrep, ones_row, qn_row, start=True, stop=True)
        qrep = work.tile([64, 64], f32, tag="qrep", name="qrep")
        nc.vector.tensor_copy(out=qrep, in_=ps_qrep)

        ps_cross = ps_a.tile([64, 64], f32, tag="scrA", name="ps_cross")
        fQ3 = flatQ.rearrange("p (i c) -> p c i", c=64)
        fK3 = flatK.rearrange("p (i c) -> p c i", c=64)
        for c in range(64):
            nc.tensor.matmul(ps_cross, fK3[:, c, :], fQ3[:, c, :],
                             start=(c == 0), stop=(c == 63))
        Et = work.tile([64, 64], f32, tag="Et", name="Et")
        nc.vector.scalar_tensor_tensor(out=Et, in0=ps_cross, scalar=-2.0,
                                       in1=qrep, op0=OP.mult, op1=OP.add)
        nc.vector.tensor_scalar(out=Et, in0=Et, scalar1=kn_col, scalar2=0.0,
                                op0=OP.add, op1=OP.max)
        lnE = work.tile([64, 64], f32, tag="lnE", name="lnE")
        nc.scalar.activation(out=lnE, in_=Et, func=ACT.Ln,
                             bias=bias_ln, scale=1.0)
        ln1 = work.tile([64, 64], f32, tag="ln1", name="ln1")
        nc.vector.tensor_scalar_add(ln1, lnE, 1.0)
        sc = work.tile([64, 64], f32, tag="sc", name="sc")
        nc.vector.reciprocal(out=sc, in_=ln1)
        expS = work.tile([64, 64], f16, tag="expS", name="expS")
        nc.scalar.activation(out=expS, in_=sc, func=ACT.Exp, bias=0.0,
                             scale=1.0)
        ps_cs = ps_a.tile([64, 1], f32, tag="scrA", name="ps_cs")
        nc.tensor.matmul(ps_cs, expS, ones_col_h, start=True, stop=True)
        inv = work.tile([64, 1], f32, tag="inv", name="inv")
        nc.vector.reciprocal(out=inv, in_=ps_cs)

        # VF: flatV [p, (i c)] -> VF [i, (p c)] via DRAM roundtrip
        VF = big.tile([64, M * 64], f16, tag="VF", name="VF")
        nc.sync.dma_start(out=scrV, in_=flatV)
        nc.sync.dma_start(
            out=VF.rearrange("i (p c) -> i p c", c=64),
            in_=scrV.rearrange("p (i c) -> i p c", c=64))
        if s == 0:
            tap("VF", VF)
        # mixing: M2[j, (p c)] = sum_i expS[i, j] VF[i, (p c)] * inv[j]
        M2 = big.tile([64, M * 64], f16, tag="M2", name="M2")
        for ch in range(4):
            ps_m = ps_b.tile([64, 1024], f32, tag="scrB", name="ps_m")
            nc.tensor.matmul(ps_m[:, 0:512], expS, VF[:, ds(ch * 1024, 512)],
                             start=True, stop=True)
            nc.tensor.matmul(ps_m[:, 512:1024], expS,
                             VF[:, ds(ch * 1024 + 512, 512)],
                             start=True, stop=True)
            nc.vector.tensor_scalar_mul(M2[:, ds(ch * 1024, 1024)], ps_m, inv)
        # S1M scatter: M2 [j=(pr h), (p c)] -> S1M [(h p), (pr c)] via DRAM
        S1M = big.tile([128, NP_PAIR * 64], f16, tag="S1M", name="S1M")
        nc.sync.dma_start(out=scrM, in_=M2)
        for h in (0, 1):
            nc.sync.dma_start(
                out=S1M[h * 64:(h + 1) * 64, :].rearrange(
                    "p (pr c) -> p pr c", c=64),
                in_=scrM.rearrange("(pr two) (p c) -> two p pr c",
                                   two=2, c=64)[h])

        if s == 0:
            tap("M2", M2)
            tap("S1M", S1M)
            tap("expS", expS)
        # ---------------- exp ----------------
        outS1 = big.tile([128, NP_PAIR * 64], f32, tag="outS1", name="outS1")
        for b in range(NBATCH):
            Xs = S1M[:, ds(b * WBS, WBS)]
            X3 = r3(Xs)
            H = chain.tile([128, WBS], f16, tag="expH", name="expH")
            rr_stt(H, Xs, 1.0 / EXP_DEG, get_aI(1.0))
            for k in range(EXP_DEG - 1, 0, -1):
                psx = ps_a.tile([128, WBS], f32, tag="scrA", name="psx")
                px3, h3 = r3(psx), r3(H)
                for hh in (0, 1):
                    hs = slice(hh * 64, hh * 64 + 64)
                    nc.tensor.matmul(psx[hs, :], get_cI(float(k))[hs, :],
                                     get_aI(1.0)[hs, :],
                                     start=True, stop=False)
                for p in range(PB):
                    for hh in (0, 1):
                        hs = slice(hh * 64, hh * 64 + 64)
                        nc.tensor.matmul(px3[hs, p], X3[hs, p], h3[hs, p],
                                         start=False, stop=(p == PB - 1),
                                         skip_group_check=True)
                H2 = chain.tile([128, WBS], f16, tag="expH", name="expH2")
                rr_copy(H2, psx, scale=1.0 / k)
                H = H2
            for sq_i in range(EXP_SQ):
                psx = ps_a.tile([128, WBS], f32, tag="scrA", name="psx2")
                px3, h3 = r3(psx), r3(H)
                for p in range(PB):
                    for hh in (0, 1):
                        hs = slice(hh * 64, hh * 64 + 64)
                        nc.tensor.matmul(px3[hs, p], h3[hs, p], h3[hs, p],
                                         start=True, stop=True)
                if sq_i < EXP_SQ - 1:
                    H2 = chain.tile([128, WBS], f16, tag="expH", name="expH3")
                    rr_copy(H2, psx)
                    H = H2
                else:
                    rr_copy(outS1[:, ds(b * WBS, WBS)], psx, scale=C_NORM)

        o3 = out_ap[s].rearrange("(pr two) r c -> two r pr c", two=2)
        nc.sync.dma_start(
            out=o3[0],
            in_=outS1[0:64, :].rearrange("p (pr c) -> p pr c", c=64))
        nc.sync.dma_start(
            out=o3[1],
            in_=outS1[64:128, :].rearrange("p (pr c) -> p pr c", c=64))


def build(nsamp=NSAMP, num_devices=NCORES, debug_taps=False):
    import concourse.bacc as bacc
    import concourse.mybir as mybir
    import concourse.tile as tile

    nc = bacc.Bacc("TRN2", target_bir_lowering=False, debug=False,
                   num_devices=num_devices)
    f32 = mybir.dt.float32
    x_ap = nc.dram_tensor("x", [nsamp, M, DIN, DIN], f32,
                          kind="ExternalInput").ap()
    wq = nc.dram_tensor("wq", [DIN, DOUT], f32, kind="ExternalInput").ap()
    wk = nc.dram_tensor("wk", [DIN, DOUT], f32, kind="ExternalInput").ap()
    wv = nc.dram_tensor("wv", [DIN, DOUT], f32, kind="ExternalInput").ap()
    out = nc.dram_tensor("out", [nsamp, M, DOUT, DOUT], f32,
                         kind="ExternalOutput").ap()

    taps = {}
    if debug_taps:
        for nm, shp, dt_ in (("initQ", [128, 2048], mybir.dt.float16),
                             ("initK", [128, 2048], mybir.dt.float16),
                             ("initV", [128, 2048], mybir.dt.float16),
                             ("flatQ", [64, 4096], mybir.dt.float16),
                             ("flatK", [64, 4096], mybir.dt.float16),
                             ("flatV", [64, 4096], mybir.dt.float16),
                             ("M2", [64, 4096], mybir.dt.float16),
                             ("VF", [64, 4096], mybir.dt.float16),
                             ("S1M", [128, 2048], mybir.dt.float16),
                             ("expS", [64, 64], mybir.dt.float16)):
            taps[nm] = nc.dram_tensor("tap_" + nm, shp, dt_,
                                      kind="ExternalOutput").ap()
    with tile.TileContext(nc) as tc, ExitStack() as ctx:
        emit_kernel(nc, tc, ctx, x_ap, wq, wk, wv, out, nsamp=nsamp,
                    taps=taps if debug_taps else None)
    nc.compile()
    return nc


_CACHED = {}


def _get_nc(nsamp):
    from concourse.bass_interp import get_hw_module
    if nsamp not in _CACHED:
        nc = build(nsamp=nsamp)
        nc.m = get_hw_module(nc.m)
        _CACHED[nsamp] = nc
    return _CACHED[nsamp]


def kernel(x, Wq, Wk, Wv):
    from concourse.bass_utils import run_bass_kernel_spmd

    bs = x.shape[0]
    nsamp = bs // NCORES
    nc = _get_nc(nsamp)
    in_maps = []
    for c in range(NCORES):
        in_maps.append({
            "x": np.ascontiguousarray(x[c * nsamp:(c + 1) * nsamp],
                                      dtype=np.float32),
            "wq": np.ascontiguousarray(Wq, dtype=np.float32),
            "wk": np.ascontiguousarray(Wk, dtype=np.float32),
            "wv": np.ascontiguousarray(Wv, dtype=np.float32),
        })
    res = run_bass_kernel_spmd(nc, in_maps, list(range(NCORES)))
    outs = [res.results[c]["out"] for c in range(NCORES)]
    full = np.concatenate(outs, axis=0)
    return full.reshape(bs * M, DOUT, DOUT).astype(np.float32)


# revision 17
# speedup vs baseline: 2.1765x; 1.1502x over previous
"""Trainium2 Bass kernel for nn_AttentionManifold (SPD manifold attention).

For each of bs*m=2048 SPD matrices X (100x100): Q/K/V = W^T X W (64x64),
logQ/K/V = matrix log, log-Euclidean attention, mixed = prob-weighted sum
of logV, out = matrix exp(mixed).

Matrix log: Newton-Schulz coupled sqrt chain, L=3 levels, with
first-order level corrections  log A = 2^L log Y_L - sum_l 2^l log W_l,
log W ~= -(I - W);  series log(Y) via deg-8/12 Paterson-Stockmeyer.

Chain scheme '2s_approx' (emulator-validated): exact-transpose pairs
(Y, Yt, Z, Zt) in fp16, P = aI + bW only (no Pt):
    W    = {lhsT=Zt_h, rhs=Y_h}              (per matrix)
    P    = aI + b psW                        (stt)
    Y'   = {lhsT=Yt_h, rhs=P_h}              = Y P
    [Yt'|Z'] = {lhsT=bd(P), rhs=[Yt|Z]}      = [P^T Yt | P^T Z]  (packed)
    Zt'  = {lhsT=Z_h, rhs=P_h}               = (P^T Z)^T bitwise
Every W is a congruence of the SPD input => fp16-stable.

exp via scaling-squaring (deg-4 Horner, 4 squarings).
Sharding: pure data parallelism, bs=32 -> 4 samples per NeuronCore.
"""
import numpy as np
from contextlib import ExitStack

C_NORM = 16.0
BS, M, DIN, DOUT = 32, 64, 100, 64
NCORES = 8
NSAMP = BS // NCORES
NP_PAIR = M // 2          # 32 pairs per sample
PB = 4                    # pairs per chain batch
NBATCH = NP_PAIR // PB    # 8

SCHED_V = [
    [(8.965874126, -13.460097634), (2.380408822, -0.250737931),
     (2.380408822, -0.250532192), (2.380408822, -0.250326648),
     (0.861964497, -0.071654452), (1.542284382, -0.519941516)],
    [(7.758850039, -8.666077201), (0.987610378, -0.093162713),
     (1.645967366, -0.5826622), (1.507505828, -0.502426376)],
    [(6.551825952, -5.22018671), (0.65339645, -0.038866921)],
]
SCHED_QK = [
    [(8.965874126, -13.460097634), (2.380408822, -0.250737931),
     (2.380408822, -0.250532192), (2.380408822, -0.250326648),
     (0.861964497, -0.071654452), (1.542284382, -0.519941516)],
    [(7.758850039, -8.666077201), (0.987610378, -0.093162713),
     (1.645967366, -0.5826622)],
    [(6.404040404, -4.899837718), (0.670769231, -0.04132838)],
]
DEG_QK = 8
DEG_V = 8
EXP_DEG = 4
EXP_SQ = 4
L = 3
WBS = PB * 64             # 256
SBW = PB * 256            # 1024


def emit_kernel(nc, tc, ctx, x_ap, wq_ap, wk_ap, wv_ap, out_ap, nsamp=NSAMP,
                taps=None):
    def tap(name, t):
        if taps is not None and name in taps:
            nc.sync.dma_start(out=taps[name], in_=t)

    import concourse.mybir as mybir
    from concourse.bass import ds, ts
    from concourse.masks import make_identity

    f32 = mybir.dt.float32
    f32r = mybir.dt.float32r
    f16 = mybir.dt.float16
    AX = mybir.AxisListType
    OP = mybir.AluOpType
    ACT = mybir.ActivationFunctionType

    # ---- engine rotation helpers ----
    _rrc = [0]
    _rrs = [0]

    from concourse.bass import MemorySpace as _MS

    def _psum(*aps):
        return any(a.space == _MS.PSUM for a in aps)

    def rr_copy(out, in_, scale=None):
        pool = ((nc.vector, nc.scalar) if _psum(out, in_)
                else (nc.vector, nc.scalar, nc.gpsimd))
        e = pool[_rrc[0] % len(pool)]
        _rrc[0] += 1
        if e is nc.scalar:
            nc.scalar.activation(out=out, in_=in_, func=ACT.Copy, bias=0.0,
                                 scale=1.0 if scale is None else float(scale))
        elif scale is None:
            e.tensor_copy(out=out, in_=in_)
        else:
            e.tensor_scalar_mul(out, in_, float(scale))

    def rr_stt(out, in0, scalar, in1):
        nc.vector.scalar_tensor_tensor(out=out, in0=in0, scalar=float(scalar),
                                       in1=in1, op0=OP.mult, op1=OP.add)

    # ---------------- pools ----------------
    const = ctx.enter_context(tc.tile_pool(name="const", bufs=1))
    work = ctx.enter_context(tc.tile_pool(name="work", bufs=2))
    big = ctx.enter_context(tc.tile_pool(name="big", bufs=1))
    chain = ctx.enter_context(tc.tile_pool(name="chain", bufs=2))
    ps_w = ctx.enter_context(tc.tile_pool(name="ps_w", bufs=1, space="PSUM"))
    ps_bc = ctx.enter_context(tc.tile_pool(name="ps_bc", bufs=2, space="PSUM"))
    ps_a = ctx.enter_context(tc.tile_pool(name="ps_a", bufs=1, space="PSUM"))
    ps_b = ctx.enter_context(tc.tile_pool(name="ps_b", bufs=1, space="PSUM"))

    # ---------------- constants ----------------
    W3f = const.tile([DIN, 3 * DOUT], f32)
    nc.sync.dma_start(out=W3f[:, 0:DOUT], in_=wq_ap)
    nc.sync.dma_start(out=W3f[:, DOUT:2 * DOUT], in_=wk_ap)
    nc.sync.dma_start(out=W3f[:, 2 * DOUT:3 * DOUT], in_=wv_ap)
    W3r = const.tile([DIN, 256], f32r)
    nc.vector.memset(W3r.bitcast(f32), 0.0)
    nc.vector.tensor_copy(out=W3r[:, 0:192], in_=W3f)
    WQKh = const.tile([DIN, 128], f16)        # f16 weights for Q/K stage-2
    nc.vector.tensor_copy(out=WQKh, in_=W3f[:, 0:128])

    IREP = const.tile([128, 64], f16)
    make_identity(nc, IREP[0:64, :])
    make_identity(nc, IREP[64:128, :])

    aI = {}

    def get_aI(val):
        val = float(val)
        if val not in aI:
            t = const.tile([128, WBS], f16, tag=f"aI{len(aI)}",
                           name=f"aI{len(aI)}")
            for p in range(PB):
                nc.vector.tensor_scalar_mul(t[:, ts(p, 64)], IREP, val)
            aI[val] = t
        return aI[val]

    for lv in SCHED_QK + SCHED_V:
        for a, b in lv:
            get_aI(a)
    for c0 in (1.0, 0.25, 0.125):
        get_aI(c0)
    I7f = const.tile([128, WBS], f32)
    for p in range(PB):
        nc.vector.tensor_scalar_mul(I7f[:, ts(p, 64)], IREP, float(2 ** L - 1))
    cI_mm = {}

    def get_cI(val):
        val = float(val)
        if val not in cI_mm:
            t = const.tile([128, 64], f16, tag=f"cImm{len(cI_mm)}",
                           name=f"cImm{len(cI_mm)}")
            nc.vector.tensor_scalar_mul(t, IREP, val)
            cI_mm[val] = t
        return cI_mm[val]

    for v in (1.0, 0.5, 1.0 / 3.0, 2.0, 3.0):
        get_cI(v)
    for lv in SCHED_QK + SCHED_V:
        for a, b in lv:
            get_cI(a / b)
    for v in (0.25, 0.2, 1.0 / 6, 1.0 / 7, 0.125, 1.0 / 9, 0.1,
              1.0 / 11, 1.0 / 12):
        get_cI(v)

    ones_col = const.tile([64, 1], f32)
    nc.vector.memset(ones_col, 1.0)
    ones_col_h = const.tile([64, 1], f16)
    nc.vector.memset(ones_col_h, 16.0)    # folds 1/16 exp prescale into inv
    ones_row = const.tile([1, 64], f32)
    nc.vector.memset(ones_row, 1.0)
    bias_ln = const.tile([64, 1], f32)
    nc.vector.memset(bias_ln, 1.0 + 64e-6)

    # DRAM scratch for partition-moving transposes (DRAM APs unconstrained)
    scrV = nc.dram_tensor("scrV", [64, M * 64], f16, kind="Internal").ap()
    scrM = nc.dram_tensor("scrM", [64, M * 64], f16, kind="Internal").ap()


    def r3(t):
        """[p, (n c)] -> [p, n, 64] view"""
        return t.rearrange("p (n c) -> p n c", c=64)

    def slot(S, f):
        """S [128, (PB,4,64)] -> slot view [128, PB, 64]"""
        return S.rearrange("p (n four c) -> p n four c", four=4, c=64)[:, :, f, :]

    # =====================================================================
    def chain_gen(cn, sched, deg, init_t, b, flat_t):
        """One chain batch: NS chain + corrections + series -> flat_t."""
        ib = r3(init_t[:, ds(b * PB * 64, PB * 64)])     # [128, PB, 64]
        S_cur = None
        Y = Yt = Z = Zt = None       # [128, PB, 64] views
        adj = False
        ACC = None

        for l in range(L):
            steps = sched[l]
            for j, (a, bc) in enumerate(steps):
                Pd = chain.tile([128, WBS], f16, tag=f"Pd{cn}", name=f"Pd{cn}")
                Pd3 = r3(Pd)
                aIt = get_aI(a)
                if j == 0:
                    src = ib if l == 0 else Y
                    rr_stt(Pd3, src, bc, r3(aIt))
                    yield
                else:
                    psW = ps_w.tile([128, WBS], f32, tag="psW", name="psW")
                    psW3 = r3(psW)
                    abI = get_cI(a / bc)
                    Ipat = get_aI(1.0)
                    for h in (0, 1):
                        hs = slice(h * 64, h * 64 + 64)
                        nc.tensor.matmul(psW[hs, :], abI[hs, :], Ipat[hs, :],
                                         start=True, stop=False)
                    for p in range(PB):
                        for h in (0, 1):
                            hs = slice(h * 64, h * 64 + 64)
                            nc.tensor.matmul(psW3[hs, p], Zt[hs, p], Y[hs, p],
                                             start=False,
                                             stop=(p == PB - 1),
                                             skip_group_check=True)
                    yield
                    nc.scalar.activation(out=Pd, in_=psW, func=ACT.Copy,
                                         bias=0.0, scale=float(bc))
                    yield
                # ---- updates ----
                psBC = ps_bc.tile([128, SBW], f32, tag="psBC", name="psBC")
                ps4 = psBC.rearrange("p (n four c) -> p n four c", four=4, c=64)
                YtP = ib if (l == 0 and j == 0) else Yt
                for p in range(PB):
                    for h in (0, 1):
                        hs = slice(h * 64, h * 64 + 64)
                        # Y' = Yt^T P ; Yt' = P^T Yt
                        nc.tensor.matmul(ps4[hs, p, 0, :], YtP[hs, p],
                                         Pd3[hs, p], start=True, stop=True)
                        nc.tensor.matmul(ps4[hs, p, 1, :], Pd3[hs, p],
                                         YtP[hs, p], start=True, stop=True)
                        if j > 0:
                            # Z' = P^T Z ; Zt' = Z^T P
                            nc.tensor.matmul(ps4[hs, p, 2, :], Pd3[hs, p],
                                             Z[hs, p], start=True, stop=True)
                            nc.tensor.matmul(ps4[hs, p, 3, :], Z[hs, p],
                                             Pd3[hs, p], start=True, stop=True)
                yield
                S_new = chain.tile([128, SBW], f16, tag=f"S{cn}", name=f"S{cn}")
                s4 = S_new.rearrange("p (n four c) -> p n four c", four=4, c=64)
                if j == 0:
                    nc.vector.tensor_copy(out=s4[:, :, 0:2, :],
                                          in_=ps4[:, :, 0:2, :])
                    Z = Pd3
                    Zt = Pd3
                    adj = False
                else:
                    nc.vector.tensor_copy(out=S_new[:, 0:SBW // 2],
                                          in_=psBC[:, 0:SBW // 2])
                    nc.scalar.activation(out=S_new[:, SBW // 2:],
                                         in_=psBC[:, SBW // 2:],
                                         func=ACT.Copy, bias=0.0, scale=1.0)
                    Z = slot(S_new, 2)
                    Zt = slot(S_new, 3)
                    adj = True
                Y = slot(S_new, 0)
                Yt = slot(S_new, 1)
                S_cur = S_new
                yield
            # ---- level end correction: psWe = Zt^T Y + Y^T Zt ----
            psWe = ps_w.tile([128, WBS], f32, tag="psW", name="psWe")
            pw3 = r3(psWe)
            for p in range(PB):
                for h in (0, 1):
                    hs = slice(h * 64, h * 64 + 64)
                    nc.tensor.matmul(pw3[hs, p], Zt[hs, p], Y[hs, p],
                                     start=True, stop=False)
                    nc.tensor.matmul(pw3[hs, p], Y[hs, p], Zt[hs, p],
                                     start=False, stop=True)
            yield
            if l == 0:
                ACC = chain.tile([128, WBS], f32, tag=f"acc{cn}",
                                 name=f"acc{cn}")
                rr_stt(ACC, psWe, -0.5, I7f)
            else:
                rr_stt(ACC, psWe, -float(2 ** l) / 2.0, ACC)
            yield
        # ================= series =================
        E = chain.tile([128, WBS], f16, tag=f"E{cn}", name=f"E{cn}")
        rr_stt(r3(E), Y, -1.0, r3(get_aI(1.0)))
        yield
        powers = {1: E}
        for k, rt in ((2, 1), (3, 2), (4, 3)):
            psE = ps_a.tile([128, WBS], f32, tag="scrA", name="psE")
            pe3 = r3(psE)
            e1 = r3(powers[1])
            ert = r3(powers[rt])
            for p in range(PB):
                for h in (0, 1):
                    hs = slice(h * 64, h * 64 + 64)
                    nc.tensor.matmul(pe3[hs, p], e1[hs, p], ert[hs, p],
                                     start=True, stop=True)
            Ek = chain.tile([128, WBS], f16, tag=f"E{k}{cn}", name=f"E{k}{cn}")
            rr_copy(Ek, psE)
            powers[k] = Ek
            yield
        E2, E3, E4 = powers[2], powers[3], powers[4]

        def combo(coefs, dst_tag):
            """PE-accumulated c0 I + c1 E + c2 E2 + c3 E3 + c4 E4 -> f16."""
            psC = ps_a.tile([128, WBS], f32, tag="scrA", name="psC")
            ops = [(get_cI(coefs[0]), get_aI(1.0))] + [
                (get_cI(cv), pw) for cv, pw in
                zip(coefs[1:], (E, E2, E3, E4)) if cv]
            for i, (lh, rh) in enumerate(ops):
                for h in (0, 1):
                    hs = slice(h * 64, h * 64 + 64)
                    nc.tensor.matmul(psC[hs, :], lh[hs, :], rh[hs, :],
                                     start=(i == 0),
                                     stop=(i == len(ops) - 1),
                                     skip_group_check=(i > 0))
            Ct = chain.tile([128, WBS], f16, tag=dst_tag, name=dst_tag)
            rr_copy(Ct, psC)
            return Ct

        C = combo((0.25, 0.2, 1.0 / 6, 1.0 / 7, 0.125), f"C{cn}")
        yield
        if deg == 12:
            C2 = combo((0.125, 1.0 / 9, 0.1, 1.0 / 11, 1.0 / 12), f"C2{cn}")
            psH = ps_a.tile([128, WBS], f32, tag="scrA", name="psH")
            ph3, e43, c23 = r3(psH), r3(E4), r3(C2)
            for p in range(PB):
                for h in (0, 1):
                    hs = slice(h * 64, h * 64 + 64)
                    nc.tensor.matmul(ph3[hs, p], e43[hs, p], c23[hs, p],
                                     start=True, stop=True)
            yield
            CH = chain.tile([128, WBS], f16, tag=f"C2{cn}", name=f"CH{cn}")
            nc.vector.tensor_tensor(out=CH, in0=psH, in1=C, op=OP.add)
            C = CH
            yield
        # psB0 = 1*E + E4@C + (1/2)E2 + (1/3)E3   (accumulated group)
        psB0 = ps_a.tile([128, WBS], f32, tag="scrA", name="psB0")
        e43, c3 = r3(E4), r3(C)
        for h in (0, 1):
            hs = slice(h * 64, h * 64 + 64)
            nc.tensor.matmul(psB0[hs, :], get_cI(1.0)[hs, :], E[hs, :],
                             start=True, stop=False)
        for p in range(PB):
            for h in (0, 1):
                hs = slice(h * 64, h * 64 + 64)
                nc.tensor.matmul(r3(psB0)[hs, p], e43[hs, p], c3[hs, p],
                                 start=False, stop=False,
                                 skip_group_check=True)
        for h in (0, 1):
            hs = slice(h * 64, h * 64 + 64)
            nc.tensor.matmul(psB0[hs, :], get_cI(0.5)[hs, :], E2[hs, :],
                             start=False, stop=False, skip_group_check=True)
            nc.tensor.matmul(psB0[hs, :], get_cI(1.0 / 3.0)[hs, :], E3[hs, :],
                             start=False, stop=True, skip_group_check=True)
        yield
        # LS = -2^L psB0 + ACC -> flat (strided, per h)
        fl3 = flat_t.rearrange("p (pr two c) -> p pr two c", two=2, c=64)
        acc3 = r3(ACC)
        for h in (0, 1):
            hs = slice(h * 64, h * 64 + 64)
            rr_stt(fl3[:, ds(b * PB, PB), h, :], r3(psB0)[hs], -float(2 ** L),
                   acc3[hs])
        yield

    # ======================= per-sample pipeline =========================
    for s in range(nsamp):
        initQ = work.tile([128, NP_PAIR * 64], f16, tag="initQ", name="initQ")
        initK = work.tile([128, NP_PAIR * 64], f16, tag="initK", name="initK")
        initV = work.tile([128, NP_PAIR * 64], f16, tag="initV", name="initV")

        # ---------------- congruence ----------------
        for g in range(8):          # 8 matrices per group
            if g % 2 == 0:
                xbuf = work.tile([DIN, 16 * DIN], f32r, tag="xbuf", name="xbuf")
                nc.gpsimd.dma_start(
                    out=xbuf.rearrange("p (i c) -> p i c", c=DIN),
                    in_=x_ap[s, ds(g * 8, 16)].rearrange("i p c -> p i c"))
            pqks = []
            pvs = []
            for r in range(2):      # 2 rounds x 4 matrices
                ps1 = ps_b.tile([DIN, 4 * 256], f32, tag="scrB", name="ps1")
                for mi in range(4):
                    mg = (g % 2) * 8 + r * 4 + mi
                    nc.tensor.matmul(ps1[:, ts(mi, 256)],
                                     xbuf[:, ts(mg, DIN)], W3r,
                                     start=True, stop=True)
                pqk = work.tile([DIN, 4 * 128], f16, tag="pqk", name="pqk")
                pv = work.tile([DIN, 4 * 64], f32, tag="pv", name="pv")
                rr_copy(pqk.rearrange("p (n c) -> p n c", c=128),
                        ps1.rearrange("p (n c) -> p n c", c=256)[:, :, 0:128])
                rr_copy(pv.rearrange("p (n c) -> p n c", c=64),
                        ps1.rearrange("p (n c) -> p n c", c=256)[:, :, 128:192])
                pqks.append(pqk)
                pvs.append(pv)
            for wi, init_t in ((0, initQ), (1, initK), (2, initV)):
                psI = ps_w.tile([128, WBS], f32, tag="psW", name="psI")
                for m in range(8):
                    r, mi = m // 4, m % 4
                    pr, h = m // 2, m % 2
                    hs = slice(h * 64, h * 64 + 64)
                    if wi < 2:
                        rhs = pqks[r][:, mi * 128 + wi * 64:
                                      mi * 128 + wi * 64 + 64]
                    else:
                        rhs = pvs[r][:, ts(mi, 64)]
                    lhsW = (WQKh[:, ts(wi, 64)] if wi < 2
                            else W3f[:, ts(2, 64)])
                    nc.tensor.matmul(psI[hs, ts(pr, 64)], lhsW, rhs,
                                     start=True, stop=True)
                rr_copy(init_t[:, ds(g * 4 * 64, WBS)], psI, scale=1.0 / C_NORM)

        if s == 0:
            tap("initQ", initQ)
            tap("initK", initK)
            tap("initV", initV)
        # ---------------- chains ----------------
        flatQ = big.tile([64, M * 64], f16, tag="flatQ", name="flatQ")
        flatK = big.tile([64, M * 64], f16, tag="flatK", name="flatK")
        flatV = big.tile([64, M * 64], f16, tag="flatV", name="flatV")
        for b in range(NBATCH):
            gens = [chain_gen("q", SCHED_QK, DEG_QK, initQ, b, flatQ),
                    chain_gen("k", SCHED_QK, DEG_QK, initK, b, flatK),
                    chain_gen("v", SCHED_V, DEG_V, initV, b, flatV)]
            next(gens[0], None)
            next(gens[0], None)
            next(gens[1], None)
            while gens:
                gens = [g for g in gens
                        if next(g, StopIteration) is not StopIteration]

        if s == 0:
            tap("flatQ", flatQ)
            tap("flatK", flatK)
            tap("flatV", flatV)
        # ---------------- attention ----------------
        partQ = work.tile([64, M], f32, tag="partQ", name="partQ")
        partK = work.tile([64, M], f32, tag="partK", name="partK")
        for flat_t, part_t in ((flatQ, partQ), (flatK, partK)):
            sq = big.tile([64, M * 64], f16, tag="sqscr", name="sqscr")
            nc.vector.tensor_mul(sq, flat_t, flat_t)
            nc.vector.tensor_reduce(
                out=part_t, in_=sq.rearrange("p (i c) -> p i c", c=64),
                axis=AX.X, op=OP.add)
        ps_qn = ps_a.tile([1, 64], f32, tag="scrA", name="ps_qn")
        nc.tensor.matmul(ps_qn, ones_col, partQ, start=True, stop=True)
        qn_row = work.tile([1, 64], f32, tag="qnrow", name="qnrow")
        nc.vector.tensor_copy(out=qn_row, in_=ps_qn)
        ps_kn = ps_a.tile([64, 1], f32, tag="scrA", name="ps_kn")
        nc.tensor.matmul(ps_kn, partK, ones_col, start=True, stop=True)
        kn_col = work.tile([64, 1], f32, tag="kncol", name="kncol")
        nc.vector.tensor_copy(out=kn_col, in_=ps_kn)
        ps_qrep = ps_a.tile([64, 64], f32, tag="scrA", name="ps_qrep")
        nc.tensor.matmul(ps_qrep, ones_row, qn_row, start=True, stop=True)
        qrep = work.tile([64, 64], f32, tag="qrep", name="qrep")
        nc.vector.tensor_copy(out=qrep, in_=ps_qrep)

        ps_cross = ps_a.tile([64, 64], f32, tag="scrA", name="ps_cross")
        fQ3 = flatQ.rearrange("p (i c) -> p c i", c=64)
        fK3 = flatK.rearrange("p (i c) -> p c i", c=64)
        for c in range(64):
            nc.tensor.matmul(ps_cross, fK3[:, c, :], fQ3[:, c, :],
                             start=(c == 0), stop=(c == 63))
        Et = work.tile([64, 64], f32, tag="Et", name="Et")
        nc.vector.scalar_tensor_tensor(out=Et, in0=ps_cross, scalar=-2.0,
                                       in1=qrep, op0=OP.mult, op1=OP.add)
        nc.vector.tensor_scalar(out=Et, in0=Et, scalar1=kn_col, scalar2=0.0,
                                op0=OP.add, op1=OP.max)
        lnE = work.tile([64, 64], f32, tag="lnE", name="lnE")
        nc.scalar.activation(out=lnE, in_=Et, func=ACT.Ln,
                             bias=bias_ln, scale=1.0)
        ln1 = work.tile([64, 64], f32, tag="ln1", name="ln1")
        nc.vector.tensor_scalar_add(ln1, lnE, 1.0)
        sc = work.tile([64, 64], f32, tag="sc", name="sc")
        nc.vector.reciprocal(out=sc, in_=ln1)
        expS = work.tile([64, 64], f16, tag="expS", name="expS")
        nc.scalar.activation(out=expS, in_=sc, func=ACT.Exp, bias=0.0,
                             scale=1.0)
        ps_cs = ps_a.tile([64, 1], f32, tag="scrA", name="ps_cs")
        nc.tensor.matmul(ps_cs, expS, ones_col_h, start=True, stop=True)
        inv = work.tile([64, 1], f32, tag="inv", name="inv")
        nc.vector.reciprocal(out=inv, in_=ps_cs)

        # VF: flatV [p, (i c)] -> VF [i, (p c)] via DRAM roundtrip
        VF = big.tile([64, M * 64], f16, tag="VF", name="VF")
        nc.sync.dma_start(out=scrV, in_=flatV)
        nc.sync.dma_start(
            out=VF.rearrange("i (p c) -> i p c", c=64),
            in_=scrV.rearrange("p (i c) -> i p c", c=64))
        if s == 0:
            tap("VF", VF)
        # mixing: M2[j, (p c)] = sum_i expS[i, j] VF[i, (p c)] * inv[j]
        M2 = big.tile([64, M * 64], f16, tag="M2", name="M2")
        for ch in range(4):
            ps_m = ps_b.tile([64, 1024], f32, tag="scrB", name="ps_m")
            nc.tensor.matmul(ps_m[:, 0:512], expS, VF[:, ds(ch * 1024, 512)],
                             start=True, stop=True)
            nc.tensor.matmul(ps_m[:, 512:1024], expS,
                             VF[:, ds(ch * 1024 + 512, 512)],
                             start=True, stop=True)
            nc.vector.tensor_scalar_mul(M2[:, ds(ch * 1024, 1024)], ps_m, inv)
        # S1M scatter: M2 [j=(pr h), (p c)] -> S1M [(h p), (pr c)] via DRAM
        S1M = big.tile([128, NP_PAIR * 64], f16, tag="S1M", name="S1M")
        nc.sync.dma_start(out=scrM, in_=M2)
        for h in (0, 1):
            nc.sync.dma_start(
                out=S1M[h * 64:(h + 1) * 64, :].rearrange(
                    "p (pr c) -> p pr c", c=64),
                in_=scrM.rearrange("(pr two) (p c) -> two p pr c",
                                   two=2, c=64)[h])

        if s == 0:
            tap("M2", M2)
            tap("S1M", S1M)
            tap("expS", expS)
        # ---------------- exp ----------------
        outS1 = big.tile([128, NP_PAIR * 64], f32, tag="outS1", name="outS1")
        for b in range(NBATCH):
            Xs = S1M[:, ds(b * WBS, WBS)]
            X3 = r3(Xs)
            H = chain.tile([128, WBS], f16, tag="expH", name="expH")
            rr_stt(H, Xs, 1.0 / EXP_DEG, get_aI(1.0))
            for k in range(EXP_DEG - 1, 0, -1):
                psx = ps_a.tile([128, WBS], f32, tag="scrA", name="psx")
                px3, h3 = r3(psx), r3(H)
                for hh in (0, 1):
                    hs = slice(hh * 64, hh * 64 + 64)
                    nc.tensor.matmul(psx[hs, :], get_cI(float(k))[hs, :],
                                     get_aI(1.0)[hs, :],
                                     start=True, stop=False)
                for p in range(PB):
                    for hh in (0, 1):
                        hs = slice(hh * 64, hh * 64 + 64)
                        nc.tensor.matmul(px3[hs, p], X3[hs, p], h3[hs, p],
                                         start=False, stop=(p == PB - 1),
                                         skip_group_check=True)
                H2 = chain.tile([128, WBS], f16, tag="expH", name="expH2")
                rr_copy(H2, psx, scale=1.0 / k)
                H = H2
            for sq_i in range(EXP_SQ):
                psx = ps_a.tile([128, WBS], f32, tag="scrA", name="psx2")
                px3, h3 = r3(psx), r3(H)
                for p in range(PB):
                    for hh in (0, 1):
                        hs = slice(hh * 64, hh * 64 + 64)
                        nc.tensor.matmul(px3[hs, p], h3[hs, p], h3[hs, p],
                                         start=True, stop=True)
                if sq_i < EXP_SQ - 1:
                    H2 = chain.tile([128, WBS], f16, tag="expH", name="expH3")
                    rr_copy(H2, psx)
                    H = H2
                else:
                    rr_copy(outS1[:, ds(b * WBS, WBS)], psx, scale=C_NORM)

        o3 = out_ap[s].rearrange("(pr two) r c -> two r pr c", two=2)
        nc.sync.dma_start(
            out=o3[0],
            in_=outS1[0:64, :].rearrange("p (pr c) -> p pr c", c=64))
        nc.sync.dma_start(
            out=o3[1],
            in_=outS1[64:128, :].rearrange("p (pr c) -> p pr c", c=64))


def build(nsamp=NSAMP, num_devices=NCORES, debug_taps=False):
    import concourse.bacc as bacc
    import concourse.mybir as mybir
    import concourse.tile as tile

    nc = bacc.Bacc("TRN2", target_bir_lowering=False, debug=False,
                   num_devices=num_devices)
    f32 = mybir.dt.float32
    x_ap = nc.dram_tensor("x", [nsamp, M, DIN, DIN], f32,
                          kind="ExternalInput").ap()
    wq = nc.dram_tensor("wq", [DIN, DOUT], f32, kind="ExternalInput").ap()
    wk = nc.dram_tensor("wk", [DIN, DOUT], f32, kind="ExternalInput").ap()
    wv = nc.dram_tensor("wv", [DIN, DOUT], f32, kind="ExternalInput").ap()
    out = nc.dram_tensor("out", [nsamp, M, DOUT, DOUT], f32,
                         kind="ExternalOutput").ap()

    taps = {}
    if debug_taps:
        for nm, shp, dt_ in (("initQ", [128, 2048], mybir.dt.float16),
                             ("initK", [128, 2048], mybir.dt.float16),
                             ("initV", [128, 2048], mybir.dt.float16),
                             ("flatQ", [64, 4096], mybir.dt.float16),
                             ("flatK", [64, 4096], mybir.dt.float16),
                             ("flatV", [64, 4096], mybir.dt.float16),
                             ("M2", [64, 4096], mybir.dt.float16),
                             ("VF", [64, 4096], mybir.dt.float16),
                             ("S1M", [128, 2048], mybir.dt.float16),
                             ("expS", [64, 64], mybir.dt.float16)):
            taps[nm] = nc.dram_tensor("tap_" + nm, shp, dt_,
                                      kind="ExternalOutput").ap()
    with tile.TileContext(nc) as tc, ExitStack() as ctx:
        emit_kernel(nc, tc, ctx, x_ap, wq, wk, wv, out, nsamp=nsamp,
                    taps=taps if debug_taps else None)
    nc.compile()
    return nc


_CACHED = {}


def _get_nc(nsamp):
    from concourse.bass_interp import get_hw_module
    if nsamp not in _CACHED:
        nc = build(nsamp=nsamp)
        nc.m = get_hw_module(nc.m)
        _CACHED[nsamp] = nc
    return _CACHED[nsamp]


def kernel(x, Wq, Wk, Wv):
    from concourse.bass_utils import run_bass_kernel_spmd

    bs = x.shape[0]
    nsamp = bs // NCORES
    nc = _get_nc(nsamp)
    in_maps = []
    for c in range(NCORES):
        in_maps.append({
            "x": np.ascontiguousarray(x[c * nsamp:(c + 1) * nsamp],
                                      dtype=np.float32),
            "wq": np.ascontiguousarray(Wq, dtype=np.float32),
            "wk": np.ascontiguousarray(Wk, dtype=np.float32),
            "wv": np.ascontiguousarray(Wv, dtype=np.float32),
        })
    res = run_bass_kernel_spmd(nc, in_maps, list(range(NCORES)))
    outs = [res.results[c]["out"] for c in range(NCORES)]
    full = np.concatenate(outs, axis=0)
    return full.reshape(bs * M, DOUT, DOUT).astype(np.float32)
